# revision 5
# baseline (speedup 1.0000x reference)
"""Trainium2 Bass kernel for nn_BoxCorrelation (epipolar point/box correlation).

Strategy
--------
Shard the N=20000 points across 8 NeuronCores (2500 each, padded to 2560 =
20 tiles of 128).  Per 128-point tile (partitions = points):

  1. one-hot(img_id) features f = [oh*x | oh*y | oh]  ->  PE transpose -> f_T
  2. PE matmuls gather the per-point 4x4 view transforms (Mg) and directly
     produce the per-(point, view, box-boundary) line coefficients A, B with
     the box boundaries folded into host-precomputed weight tables.
  3. DVE computes the projective points xy = (Mg.[x*d,y*d,d,1])/max(z,eps),
     validity mask, and the contiguous valid-depth interval [LM, HM).
  4. Box tests use monotonicity of the epipolar curve in depth: each boundary
     condition is a half-line in depth, converted to an exact LID-grid index
     count via an approximate ACT sqrt + an exact f32 compare correction
     (w >= (2h+1)^2), so results are bit-identical to the reference
     comparisons up to ulp-level input rounding.
  5. corr = (max of interval lower bounds) < (min of upper bounds).

Outputs per core: xy [2560,384] f32, mask [2560,192] u8, corr [2560,384] u8,
gathered and reshaped on host.
"""

import numpy as np

import concourse.bass as bass
import concourse.bacc as bacc
import concourse.tile as tile
from concourse import mybir
from concourse.bass_utils import run_bass_kernel_spmd

# problem dims (hardcoded per spec)
N, V, P, D = 20000, 6, 64, 32
W2D, H2D = 1600.0, 928.0
NCORES = 8
NL = N // NCORES            # 2500 points per core
TP = 128                    # points per tile
NT = (NL + TP - 1) // TP    # 20 tiles
NLP = NT * TP               # 2560 padded points per core

FB = 4 * V * P              # 1536: (boundary, view, box) free width
FVD = V * D                 # 192
FVD2 = 2 * FVD              # 384
FP = V * P                  # 384

# LID depth-grid constants (match reference f32 arithmetic exactly)
S32 = np.float32(0.5)
BIN = np.float32(np.float32(70.0 - 0.5) / np.float32(D * (D + 1)))
INVBIN = np.float32(np.float32(1.0) / BIN)
C1 = np.float32(np.float32(4.0) * INVBIN)
C2 = np.float32(np.float32(1.0) - np.float32(4.0) * S32 * INVBIN)
EPS = np.float32(1e-5)

F32 = mybir.dt.float32
I32 = mybir.dt.int32
U8 = mybir.dt.uint8
Alu = mybir.AluOpType

# consts layout (columns of the replicated const tile)
_CG = 0            # 6: 0..5 view ids
_CD = 6            # 32: depth values
_CD33 = 38         # 32: d + 33
_CD1 = 70          # 32: d + 1
_CWH = 102         # 2: [W, H]
_C32 = 104         # 1: 32.0
_CZERO = 105       # 1: 0.0
CW = 106

_CACHE = {}


def _ap(base, pattern):
    """AP with base's partition dim + custom free [step, count] pattern."""
    return bass.AP(tensor=base.tensor, offset=base.offset,
                   ap=[base.ap[0]] + [list(p) for p in pattern])


def _build_nc(reps=1):
    nc = bacc.Bacc(None, target_bir_lowering=False)

    pts_d = nc.dram_tensor("pts", [NLP, 3], F32, kind="ExternalInput")
    wtr_d = nc.dram_tensor("wtr", [6, 72], F32, kind="ExternalInput")
    rhsA_d = nc.dram_tensor("rhsA", [18, FB], F32, kind="ExternalInput")
    rhsB_d = nc.dram_tensor("rhsB", [18, FB], F32, kind="ExternalInput")
    consts_d = nc.dram_tensor("consts", [128, CW], F32, kind="ExternalInput")
    eye_d = nc.dram_tensor("eye", [128, 128], F32, kind="ExternalInput")

    xy_d = nc.dram_tensor("xy", [NLP, FVD2], F32, kind="ExternalOutput")
    mask_d = nc.dram_tensor("masko", [NLP, FVD], U8, kind="ExternalOutput")
    corr_d = nc.dram_tensor("corro", [NLP, FP], U8, kind="ExternalOutput")

    Vv = nc.vector

    with tile.TileContext(nc) as tc:
        with (
            tc.tile_pool(name="cst", bufs=1) as cst,
            tc.tile_pool(name="io", bufs=3) as io,
            tc.tile_pool(name="wk", bufs=2) as wk,
            tc.tile_pool(name="big", bufs=1) as big,
            tc.tile_pool(name="ps_t", bufs=1, space="PSUM") as ps_t,
            tc.tile_pool(name="ps_mg", bufs=1, space="PSUM") as ps_mg,
            tc.tile_pool(name="ps_a", bufs=1, space="PSUM") as ps_a,
            tc.tile_pool(name="ps_b", bufs=1, space="PSUM") as ps_b,
        ):
            cn = cst.tile([128, CW], F32)
            eye = cst.tile([128, 128], F32)
            wtr = cst.tile([6, 72], F32)
            rhsA = cst.tile([18, FB], F32)
            rhsB = cst.tile([18, FB], F32)
            nc.sync.dma_start(out=cn, in_=consts_d[:, :])
            nc.sync.dma_start(out=eye, in_=eye_d[:, :])
            nc.sync.dma_start(out=wtr, in_=wtr_d[:, :])
            nc.sync.dma_start(out=rhsA, in_=rhsA_d[:, :])
            nc.sync.dma_start(out=rhsB, in_=rhsB_d[:, :])

            for rep in range(reps):
             for it in range(NT):
                r0 = it * TP
                ptst = io.tile([TP, 3], F32, tag="ptst")
                nc.sync.dma_start(out=ptst, in_=pts_d[r0:r0 + TP, :])
                id_c = ptst[:, 0:1]
                x_c = ptst[:, 1:2]
                y_c = ptst[:, 2:3]

                # ---- features f = [oh*x | oh*y | oh], then transpose ----
                f = wk.tile([TP, 18], F32, tag="f")
                Vv.tensor_scalar(out=f[:, 0:6], in0=cn[:, _CG:_CG + 6],
                                 scalar1=id_c, scalar2=None, op0=Alu.is_equal)
                Vv.tensor_scalar(out=f[:, 6:12], in0=f[:, 0:6],
                                 scalar1=x_c, scalar2=None, op0=Alu.mult)
                Vv.tensor_scalar(out=f[:, 12:18], in0=f[:, 0:6],
                                 scalar1=y_c, scalar2=None, op0=Alu.mult)
                fT_ps = ps_t.tile([18, TP], F32, tag="fTp")
                nc.tensor.transpose(fT_ps[:, :], f[:, :], eye[:, :])
                fT = wk.tile([18, TP], F32, tag="fT")
                nc.scalar.copy(out=fT, in_=fT_ps)

                # ---- gather per-point transforms: Mg[n,(v,i,j)] ----
                mg_ps = ps_mg.tile([TP, 72], F32, tag="mgp")
                nc.tensor.matmul(mg_ps[:, :], fT[0:6, :], wtr[:, :])
                mg = wk.tile([TP, 72], F32, tag="mg")
                nc.scalar.copy(out=mg, in_=mg_ps)

                # ---- box-boundary line coefficients A, B  [128, 1536] ----
                a_ps = ps_a.tile([TP, FB], F32, tag="aps")
                b_ps = ps_b.tile([TP, FB], F32, tag="bps")
                for c0 in range(0, FB, 512):
                    nc.tensor.matmul(a_ps[:, c0:c0 + 512], fT[:, :],
                                     rhsA[:, c0:c0 + 512])
                    nc.tensor.matmul(b_ps[:, c0:c0 + 512], fT[:, :],
                                     rhsB[:, c0:c0 + 512])

                # ---- projection P_all[n,(c,v,d)] = Mg . [x*d, y*d, d, 1] ----
                xdyd = wk.tile([TP, 2 * D], F32, tag="xdyd")
                Vv.tensor_scalar(out=xdyd[:, 0:D], in0=cn[:, _CD:_CD + D],
                                 scalar1=x_c, scalar2=None, op0=Alu.mult)
                Vv.tensor_scalar(out=xdyd[:, D:2 * D], in0=cn[:, _CD:_CD + D],
                                 scalar1=y_c, scalar2=None, op0=Alu.mult)

                pall = big.tile([TP, 3 * FVD], F32, tag="pall")
                ptmp = big.tile([TP, 3 * FVD], F32, tag="ptmp")
                pall_a = _ap(pall, [[FVD, 3], [D, 6], [1, D]])
                ptmp_a = _ap(ptmp, [[FVD, 3], [D, 6], [1, D]])
                xd_b = _ap(xdyd[:, 0:1], [[0, 3], [0, 6], [1, D]])
                yd_b = _ap(xdyd[:, D:D + 1], [[0, 3], [0, 6], [1, D]])
                dep_b = _ap(cn[:, _CD:_CD + 1], [[0, 3], [0, 6], [1, D]])

                def mg_b(j):
                    return _ap(mg[:, j:j + 1], [[4, 3], [12, 6], [0, D]])

                Vv.tensor_tensor(out=pall_a, in0=xd_b, in1=mg_b(0), op=Alu.mult)
                Vv.tensor_tensor(out=ptmp_a, in0=yd_b, in1=mg_b(1), op=Alu.mult)
                Vv.tensor_tensor(out=pall_a, in0=pall_a, in1=ptmp_a, op=Alu.add)
                Vv.tensor_tensor(out=ptmp_a, in0=dep_b, in1=mg_b(2), op=Alu.mult)
                Vv.tensor_tensor(out=pall_a, in0=pall_a, in1=ptmp_a, op=Alu.add)
                Vv.tensor_tensor(out=pall_a, in0=pall_a, in1=mg_b(3), op=Alu.add)

                # ---- xy = P/max(z,eps); validity mask; [LM,HM) interval ----
                zc = wk.tile([TP, FVD], F32, tag="zc")
                Vv.tensor_scalar(out=zc, in0=pall[:, 2 * FVD:3 * FVD],
                                 scalar1=float(EPS), scalar2=None, op0=Alu.max)
                rz = wk.tile([TP, FVD], F32, tag="rz")
                Vv.reciprocal(out=rz, in_=zc)

                xyt = io.tile([TP, FVD2], F32, tag="xyt")
                Vv.tensor_tensor(out=_ap(xyt, [[2, FVD]]),
                                 in0=pall[:, 0:FVD], in1=rz, op=Alu.mult)
                Vv.tensor_tensor(out=_ap(xyt[:, 1:2], [[2, FVD]]),
                                 in0=pall[:, FVD:2 * FVD], in1=rz, op=Alu.mult)

                mt1 = big.tile([TP, FVD2], F32, tag="mt1")
                mt2 = big.tile([TP, FVD2], F32, tag="mt2")
                Vv.tensor_scalar(out=mt1, in0=xyt, scalar1=0.0, scalar2=None,
                                 op0=Alu.is_ge)
                wh_b = _ap(cn[:, _CWH:_CWH + 1], [[0, FVD], [1, 2]])
                Vv.tensor_tensor(out=mt2, in0=xyt, in1=wh_b, op=Alu.is_lt)
                Vv.tensor_tensor(out=mt1, in0=mt1, in1=mt2, op=Alu.mult)
                maskf = wk.tile([TP, FVD], F32, tag="maskf")
                Vv.tensor_tensor(out=maskf, in0=_ap(mt1, [[2, FVD]]),
                                 in1=_ap(mt1[:, 1:2], [[2, FVD]]), op=Alu.mult)
                mzt = wk.tile([TP, FVD], F32, tag="mzt")
                Vv.tensor_scalar(out=mzt, in0=pall[:, 2 * FVD:3 * FVD],
                                 scalar1=float(EPS), scalar2=None, op0=Alu.is_gt)
                Vv.tensor_tensor(out=maskf, in0=maskf, in1=mzt, op=Alu.mult)

                mask_u8 = io.tile([TP, FVD], U8, tag="mask_u8")
                Vv.tensor_copy(out=mask_u8, in_=maskf)
                nc.sync.dma_start(out=mask_d[r0:r0 + TP, :], in_=mask_u8)

                # LM = min_d(mask ? d : >=33), HM = max_d(mask ? d+1 : 0)
                lmt = wk.tile([TP, FVD], F32, tag="lmt")
                Vv.tensor_scalar(out=lmt, in0=maskf, scalar1=-33.0,
                                 scalar2=None, op0=Alu.mult)
                d33_b = _ap(cn[:, _CD33:_CD33 + 1], [[0, 6], [1, D]])
                Vv.tensor_tensor(out=_ap(lmt, [[D, 6], [1, D]]),
                                 in0=_ap(lmt, [[D, 6], [1, D]]), in1=d33_b,
                                 op=Alu.add)
                lm = wk.tile([TP, 6], F32, tag="lm")
                Vv.tensor_reduce(out=lm, in_=_ap(lmt, [[D, 6], [1, D]]),
                                 axis=mybir.AxisListType.X, op=Alu.min)
                hmt = wk.tile([TP, FVD], F32, tag="hmt")
                d1_b = _ap(cn[:, _CD1:_CD1 + 1], [[0, 6], [1, D]])
                Vv.tensor_tensor(out=_ap(hmt, [[D, 6], [1, D]]), in0=_ap(
                    maskf, [[D, 6], [1, D]]), in1=d1_b, op=Alu.mult)
                hm = wk.tile([TP, 6], F32, tag="hm")
                Vv.tensor_reduce(out=hm, in_=_ap(hmt, [[D, 6], [1, D]]),
                                 axis=mybir.AxisListType.X, op=Alu.max)

                # ---- box boundary counts (exact via w >= (2h+1)^2) ----
                rA = big.tile([TP, FB], F32, tag="rA")
                Vv.reciprocal(out=rA, in_=a_ps)
                tb = big.tile([TP, FB], F32, tag="tb")
                Vv.tensor_tensor(out=tb, in0=b_ps, in1=rA, op=Alu.mult)
                wv = big.tile([TP, FB], F32, tag="wv")
                Vv.tensor_scalar(out=wv, in0=tb, scalar1=float(C1),
                                 scalar2=float(C2), op0=Alu.mult, op1=Alu.add)
                wc = big.tile([TP, FB], F32, tag="wc")
                Vv.tensor_scalar(out=wc, in0=wv, scalar1=0.0, scalar2=1e6,
                                 op0=Alu.max, op1=Alu.min)
                q = big.tile([TP, FB], F32, tag="q")
                nc.scalar.activation(out=q, in_=wc,
                                     func=mybir.ActivationFunctionType.Sqrt)
                hp = big.tile([TP, FB], F32, tag="hp")
                Vv.tensor_scalar(out=hp, in0=q, scalar1=1.0, scalar2=0.5,
                                 op0=Alu.subtract, op1=Alu.mult)
                h32 = big.tile([TP, FB], I32, tag="h32")
                Vv.tensor_copy(out=h32, in_=hp)
                hf = big.tile([TP, FB], F32, tag="hf")
                Vv.tensor_copy(out=hf, in_=h32)
                oh2 = big.tile([TP, FB], F32, tag="oh2")
                Vv.tensor_scalar(out=oh2, in0=hf, scalar1=2.0, scalar2=1.0,
                                 op0=Alu.mult, op1=Alu.add)
                Vv.tensor_tensor(out=oh2, in0=oh2, in1=oh2, op=Alu.mult)
                cge = big.tile([TP, FB], F32, tag="cge")
                Vv.tensor_tensor(out=cge, in0=wv, in1=oh2, op=Alu.is_ge)
                cnt = big.tile([TP, FB], F32, tag="cnt")
                Vv.tensor_tensor(out=cnt, in0=hf, in1=cge, op=Alu.add)

                s8 = big.tile([TP, FB], U8, tag="s8")
                Vv.tensor_scalar(out=s8, in0=a_ps, scalar1=0.0, scalar2=None,
                                 op0=Alu.is_ge)
                lo = big.tile([TP, FB], F32, tag="lo")
                nc.gpsimd.memset(lo[:, :], 0.0)
                Vv.copy_predicated(out=lo, mask=s8, data=cnt)
                c32_b = _ap(cn[:, _C32:_C32 + 1], [[0, FB]])
                Vv.copy_predicated(out=cnt, mask=s8, data=c32_b)

                # ---- combine: corr = floor-exact (LO < HI) ----
                lo2 = wk.tile([TP, 2 * FP], F32, tag="lo2")
                Vv.tensor_tensor(out=lo2, in0=lo[:, 0:2 * FP],
                                 in1=lo[:, 2 * FP:4 * FP], op=Alu.max)
                lof = wk.tile([TP, FP], F32, tag="lof")
                Vv.tensor_tensor(out=lof, in0=lo2[:, 0:FP],
                                 in1=lo2[:, FP:2 * FP], op=Alu.max)
                lm_b = _ap(lm[:, 0:1], [[1, 6], [0, P]])
                Vv.tensor_tensor(out=lof, in0=lof, in1=lm_b, op=Alu.max)

                hi2 = wk.tile([TP, 2 * FP], F32, tag="hi2")
                Vv.tensor_tensor(out=hi2, in0=cnt[:, 0:2 * FP],
                                 in1=cnt[:, 2 * FP:4 * FP], op=Alu.min)
                hif = wk.tile([TP, FP], F32, tag="hif")
                Vv.tensor_tensor(out=hif, in0=hi2[:, 0:FP],
                                 in1=hi2[:, FP:2 * FP], op=Alu.min)
                hm_b = _ap(hm[:, 0:1], [[1, 6], [0, P]])
                Vv.tensor_tensor(out=hif, in0=hif, in1=hm_b, op=Alu.min)

                corr_u8 = io.tile([TP, FP], U8, tag="corr_u8")
                Vv.tensor_tensor(out=corr_u8, in0=lof, in1=hif, op=Alu.is_lt)
                nc.sync.dma_start(out=corr_d[r0:r0 + TP, :], in_=corr_u8)
                nc.sync.dma_start(out=xy_d[r0:r0 + TP, :], in_=xyt)

    nc.finalize()
    return nc


def _host_tables(trans_mats, rois_pad):
    tm = np.asarray(trans_mats, dtype=np.float32)
    rois = np.asarray(rois_pad, dtype=np.float32)

    # wtr[g, v*12 + i*4 + j] = tm[v, g, i, j]  (i < 3)
    wtr = np.ascontiguousarray(
        tm[:, :, 0:3, :].transpose(1, 0, 2, 3)).reshape(6, 72)

    # coefficient tables: Wc[ct*6+g, v, i] = tm[v,g,i,ct]; Wb[g,v,i]=tm[v,g,i,3]
    # f rows: [0:6]=onehot, [6:12]=onehot*x, [12:18]=onehot*y
    Wc = np.zeros((18, V, 3), np.float32)
    for blk, ct in enumerate((2, 0, 1)):
        Wc[blk * 6:(blk + 1) * 6] = tm[:, :, 0:3, ct].transpose(1, 0, 2)
    Wb = tm[:, :, 0:3, 3].transpose(1, 0, 2)  # [g, v, i]

    rhsA = np.zeros((18, 4, V, P), np.float32)
    rhsB = np.zeros((18, 4, V, P), np.float32)
    specs = [(0, True, 0), (2, False, 0), (1, True, 1), (3, False, 1)]
    for bnd, (ci, gt, ai) in enumerate(specs):
        c = rois[:, :, ci]                                   # [V, P]
        for v in range(V):
            if gt:
                rhsA[:, bnd, v, :] = (Wc[:, v, ai][:, None]
                                      - c[v][None, :] * Wc[:, v, 2][:, None])
                rhsB[0:6, bnd, v, :] = (c[v][None, :] * Wb[:, v, 2][:, None]
                                          - Wb[:, v, ai][:, None])
            else:
                rhsA[:, bnd, v, :] = (c[v][None, :] * Wc[:, v, 2][:, None]
                                      - Wc[:, v, ai][:, None])
                rhsB[0:6, bnd, v, :] = (Wb[:, v, ai][:, None]
                                          - c[v][None, :] * Wb[:, v, 2][:, None])
    rhsA = rhsA.reshape(18, FB).astype(np.float32)
    rhsB = rhsB.reshape(18, FB).astype(np.float32)

    idxf = np.arange(D, dtype=np.float32)
    dep = (S32 + BIN * idxf * (idxf + 1)).astype(np.float32)
    row = np.zeros(CW, np.float32)
    row[_CG:_CG + 6] = np.arange(6, dtype=np.float32)
    row[_CD:_CD + D] = dep
    row[_CD33:_CD33 + D] = idxf + 33.0
    row[_CD1:_CD1 + D] = idxf + 1.0
    row[_CWH] = W2D
    row[_CWH + 1] = H2D
    row[_C32] = 32.0
    consts = np.tile(row[None, :], (128, 1)).astype(np.float32)
    eye = np.eye(128, dtype=np.float32)
    return wtr, rhsA, rhsB, consts, eye


def kernel(points, trans_mats, rois_pad):
    points = np.asarray(points, dtype=np.float32)
    reps = int(_CACHE.get("reps", 1))
    key = ("nc", reps)
    if key not in _CACHE:
        _CACHE[key] = _build_nc(reps)
    nc = _CACHE[key]

    wtr, rhsA, rhsB, consts, eye = _host_tables(trans_mats, rois_pad)

    in_maps = []
    for c in range(NCORES):
        shard = np.zeros((NLP, 3), np.float32)
        shard[:NL] = points[c * NL:(c + 1) * NL]
        in_maps.append({"pts": shard, "wtr": wtr, "rhsA": rhsA,
                        "rhsB": rhsB, "consts": consts, "eye": eye})

    res = run_bass_kernel_spmd(nc, in_maps, core_ids=list(range(NCORES)),
                               trace=bool(_CACHE.get("trace")),
                               tmpdir=_CACHE.get("trace_dir"))
    _CACHE["last"] = res
    outs = res.results

    xy = np.concatenate([r["xy"][:NL] for r in outs], axis=0)
    mask = np.concatenate([r["masko"][:NL] for r in outs], axis=0)
    corr = np.concatenate([r["corro"][:NL] for r in outs], axis=0)
    xy = xy.reshape(N, V, D, 2)
    mask = mask.reshape(N, V, D).astype(np.bool_)
    corr = corr.reshape(N, V, P).astype(np.bool_)
    return xy, mask, corr


# revision 7
# speedup vs baseline: 1.9274x; 1.9274x over previous
"""Trainium2 Bass kernel for nn_BoxCorrelation (epipolar point/box correlation).

Strategy
--------
Shard the N=20000 points across 8 NeuronCores (2500 each, padded to 2560).
Per tile, 128 partitions each hold G consecutive points (free dims carry the
per-point (view, box, depth) structure), so instruction count scales as
points / (128*G).

  1. one-hot(img_id) features f -> per-point-slot PE transpose -> f_T
  2. PE matmuls gather the per-point 4x4 view transforms (Mg) and produce
     the per-(point, boundary, view, box) epipolar line coefficients A, B
     with the box boundaries folded into host-precomputed weight tables.
  3. DVE computes xy = (Mg.[x*d,y*d,d,1])/max(z,eps), the validity mask,
     and the contiguous valid-depth index interval [LM, HM).
  4. Each box boundary is a half-line in depth; its exact LID-grid index
     count is recovered with an approximate ACT sqrt plus an exact f32
     compare (w >= (2h+1)^2), bit-identical to reference comparisons.
  5. corr = (max of interval lower bounds) < (min of upper bounds).
"""

import numpy as np

import concourse.bass as bass
import concourse.bacc as bacc
import concourse.tile as tile
from concourse import mybir
from concourse.bass_utils import run_bass_kernel_spmd

# problem dims (hardcoded per spec)
N, V, P, D = 20000, 6, 64, 32
W2D, H2D = 1600.0, 928.0
NCORES = 8
NL = N // NCORES            # 2500 points per core
G = 2                       # points per partition
TP = 128                    # partitions
PTS_T = TP * G              # points per tile
NT = (NL + PTS_T - 1) // PTS_T
NLP = NT * PTS_T            # padded points per core

FB = 4 * V * P              # 1536 per point: (boundary, view, box)
FVD = V * D                 # 192
FVD2 = 2 * FVD              # 384
FP = V * P                  # 384

S32 = np.float32(0.5)
BIN = np.float32(np.float32(70.0 - 0.5) / np.float32(D * (D + 1)))
INVBIN = np.float32(np.float32(1.0) / BIN)
C1 = np.float32(np.float32(4.0) * INVBIN)
C2 = np.float32(np.float32(1.0) - np.float32(4.0) * S32 * INVBIN)
EPS = np.float32(1e-5)

F32 = mybir.dt.float32
I32 = mybir.dt.int32
U8 = mybir.dt.uint8
Alu = mybir.AluOpType

# consts columns
_CG = 0            # 6: 0..5 view ids
_CD = 6            # 32: depth values
_CD33 = 38         # 32: d + 33
_CD1 = 70          # 32: d + 1
_CWH = 102         # 2: [W, H]
_C32 = 104         # 1: 32.0
CW = 106

_CACHE = {}


def _ap(base, pattern, off=0):
    """AP with base's partition dim + custom free [step, count] pattern."""
    return bass.AP(tensor=base.tensor, offset=base.offset + off,
                   ap=[base.ap[0]] + [list(p) for p in pattern])


def _dap(handle, poff, pstep, free):
    """DRAM AP: partition dim [pstep, 128] at element offset poff."""
    base = handle[:, :]
    return bass.AP(tensor=base.tensor, offset=poff,
                   ap=[[pstep, TP]] + [list(p) for p in free])


def _build_nc(reps=1):
    nc = bacc.Bacc(None, target_bir_lowering=False)

    pts_d = nc.dram_tensor("pts", [NLP, 3], F32, kind="ExternalInput")
    wtr_d = nc.dram_tensor("wtr", [6, 72], F32, kind="ExternalInput")
    rhsA_d = nc.dram_tensor("rhsA", [18, FB], F32, kind="ExternalInput")
    rhsB_d = nc.dram_tensor("rhsB", [18, FB], F32, kind="ExternalInput")
    consts_d = nc.dram_tensor("consts", [128, CW], F32, kind="ExternalInput")
    eye_d = nc.dram_tensor("eye", [128, 128], F32, kind="ExternalInput")

    xy_d = nc.dram_tensor("xy", [NLP, FVD2], F32, kind="ExternalOutput")
    mask_d = nc.dram_tensor("masko", [NLP, FVD], U8, kind="ExternalOutput")
    corr_d = nc.dram_tensor("corro", [NLP, FP], U8, kind="ExternalOutput")

    Vv = nc.vector
    Sc = nc.scalar

    with tile.TileContext(nc) as tc:
        with (
            tc.tile_pool(name="cst", bufs=1) as cst,
            tc.tile_pool(name="io", bufs=2) as io,
            tc.tile_pool(name="wk", bufs=2) as wk,
            tc.tile_pool(name="big", bufs=1) as big,
            tc.tile_pool(name="ps_t", bufs=1, space="PSUM") as ps_t,
            tc.tile_pool(name="ps_mg", bufs=1, space="PSUM") as ps_mg,
            tc.tile_pool(name="ps_a", bufs=1, space="PSUM") as ps_a,
            tc.tile_pool(name="ps_b", bufs=1, space="PSUM") as ps_b,
        ):
            cn = cst.tile([128, CW], F32)
            eye = cst.tile([128, 128], F32)
            wtr = cst.tile([6, 72], F32)
            rhsA = cst.tile([18, FB], F32)
            rhsB = cst.tile([18, FB], F32)
            nc.sync.dma_start(out=cn, in_=consts_d[:, :])
            nc.sync.dma_start(out=eye, in_=eye_d[:, :])
            nc.sync.dma_start(out=wtr, in_=wtr_d[:, :])
            nc.sync.dma_start(out=rhsA, in_=rhsA_d[:, :])
            nc.sync.dma_start(out=rhsB, in_=rhsB_d[:, :])

            for rep in range(reps):
             for it in range(NT):
                r0 = it * PTS_T       # first point row of this tile
                ptst = io.tile([TP, 3 * G], F32, tag="ptst")
                nc.sync.dma_start(
                    out=ptst, in_=_dap(pts_d, r0 * 3, 3 * G, [[1, 3 * G]]))

                # ---- features f[(pt,18)] = [oh | oh*x | oh*y] ----
                f = wk.tile([TP, 18 * G], F32, tag="f")
                grow_b = _ap(cn, [[0, G], [1, 6]], _CG)
                Vv.tensor_tensor(out=_ap(f, [[18, G], [1, 6]]), in0=grow_b,
                                 in1=_ap(ptst, [[3, G], [0, 6]]),
                                 op=Alu.is_equal)
                Vv.tensor_tensor(out=_ap(f, [[18, G], [1, 6]], 6),
                                 in0=_ap(f, [[18, G], [1, 6]]),
                                 in1=_ap(ptst, [[3, G], [0, 6]], 1),
                                 op=Alu.mult)
                Vv.tensor_tensor(out=_ap(f, [[18, G], [1, 6]], 12),
                                 in0=_ap(f, [[18, G], [1, 6]]),
                                 in1=_ap(ptst, [[3, G], [0, 6]], 2),
                                 op=Alu.mult)

                fT_ps = ps_t.tile([18, TP * G], F32, tag="fTp")
                for pt in range(G):
                    nc.tensor.transpose(fT_ps[:, pt * TP:(pt + 1) * TP],
                                        f[:, pt * 18:(pt + 1) * 18],
                                        eye[:, :])
                fT = wk.tile([18, TP * G], F32, tag="fT")
                Sc.copy(out=fT, in_=fT_ps)

                # ---- per-point transforms Mg[(pt, i, v, j)] ----
                mg_ps = ps_mg.tile([TP, 72 * G], F32, tag="mgp")
                for pt in range(G):
                    nc.tensor.matmul(mg_ps[:, pt * 72:(pt + 1) * 72],
                                     fT[0:6, pt * TP:(pt + 1) * TP],
                                     wtr[:, :])
                mg = wk.tile([TP, 72 * G], F32, tag="mg")
                Sc.copy(out=mg, in_=mg_ps)

                # ---- A, B line coefficients, staged to SBUF ----
                As = big.tile([TP, FB * G], F32, tag="As")
                Bs = big.tile([TP, FB * G], F32, tag="Bs")
                for pt in range(G):
                    a_ps = ps_a.tile([TP, FB], F32, tag="aps")
                    b_ps = ps_b.tile([TP, FB], F32, tag="bps")
                    for c0 in range(0, FB, 512):
                        nc.tensor.matmul(a_ps[:, c0:c0 + 512],
                                         fT[:, pt * TP:(pt + 1) * TP],
                                         rhsA[:, c0:c0 + 512])
                        nc.tensor.matmul(b_ps[:, c0:c0 + 512],
                                         fT[:, pt * TP:(pt + 1) * TP],
                                         rhsB[:, c0:c0 + 512])
                    Sc.copy(out=As[:, pt * FB:(pt + 1) * FB], in_=a_ps)
                    Sc.copy(out=Bs[:, pt * FB:(pt + 1) * FB], in_=b_ps)

                # ---- per-term inputs replicated over c: [x*d | y*d | d] ----
                # layout (pt, c, d), identical values in the 3 c-slices, so
                # the merged (pt,c) dim stays affine in downstream 3D APs.
                dep_b3 = _ap(cn, [[0, G], [0, 3], [1, D]], _CD)
                xdr = wk.tile([TP, 96 * G], F32, tag="xdr")
                Vv.tensor_tensor(out=_ap(xdr, [[96, G], [32, 3], [1, D]]),
                                 in0=_ap(ptst, [[3, G], [0, 3], [0, D]], 1),
                                 in1=dep_b3, op=Alu.mult)
                ydr = wk.tile([TP, 96 * G], F32, tag="ydr")
                Vv.tensor_tensor(out=_ap(ydr, [[96, G], [32, 3], [1, D]]),
                                 in0=_ap(ptst, [[3, G], [0, 3], [0, D]], 2),
                                 in1=dep_b3, op=Alu.mult)
                der = wk.tile([TP, 96 * G], F32, tag="der")
                Sc.copy(out=_ap(der, [[96, G], [32, 3], [1, D]]), in_=dep_b3)

                # ---- projection P[(pt, c, v, d)] = Mg . [x*d, y*d, d, 1] ----
                pall = big.tile([TP, 576 * G], F32, tag="pall")
                ptmp = big.tile([TP, 576 * G], F32, tag="ptmp")
                pa = _ap(pall, [[192, 3 * G], [32, 6], [1, D]])
                pb = _ap(ptmp, [[192, 3 * G], [32, 6], [1, D]])

                def mg_b(j):
                    return _ap(mg, [[24, 3 * G], [4, 6], [0, D]], j)

                _reps = (xdr, ydr, der)

                def xdc_b(c):
                    return _ap(_reps[c], [[32, 3 * G], [0, 6], [1, D]])

                Vv.tensor_tensor(out=pa, in0=xdc_b(0), in1=mg_b(0),
                                 op=Alu.mult)
                Vv.tensor_tensor(out=pb, in0=xdc_b(1), in1=mg_b(1),
                                 op=Alu.mult)
                Vv.tensor_tensor(out=pa, in0=pa, in1=pb, op=Alu.add)
                Vv.tensor_tensor(out=pb, in0=xdc_b(2), in1=mg_b(2),
                                 op=Alu.mult)
                Vv.tensor_tensor(out=pa, in0=pa, in1=pb, op=Alu.add)
                Vv.tensor_tensor(out=pa, in0=pa, in1=mg_b(3), op=Alu.add)

                # ---- xy, mask ----
                zsl = _ap(pall, [[576, G], [1, FVD]], 384)
                zc = wk.tile([TP, FVD * G], F32, tag="zc")
                zcv = _ap(zc, [[FVD, G], [1, FVD]])
                Vv.tensor_scalar(out=zcv, in0=zsl, scalar1=float(EPS),
                                 scalar2=None, op0=Alu.max)
                rz = wk.tile([TP, FVD * G], F32, tag="rz")
                rzv = _ap(rz, [[FVD, G], [1, FVD]])
                Vv.reciprocal(out=rzv, in_=zcv)

                xyt = io.tile([TP, FVD2 * G], F32, tag="xyt")
                Vv.tensor_tensor(out=_ap(xyt, [[FVD2, G], [2, FVD]]),
                                 in0=_ap(pall, [[576, G], [1, FVD]]),
                                 in1=rzv, op=Alu.mult)
                Vv.tensor_tensor(out=_ap(xyt, [[FVD2, G], [2, FVD]], 1),
                                 in0=_ap(pall, [[576, G], [1, FVD]], 192),
                                 in1=rzv, op=Alu.mult)

                mt1 = big.tile([TP, FVD2 * G], F32, tag="mt1")
                mt2 = big.tile([TP, FVD2 * G], F32, tag="mt2")
                Vv.tensor_scalar(out=mt1, in0=xyt, scalar1=0.0, scalar2=None,
                                 op0=Alu.is_ge)
                wh_b = _ap(cn, [[0, G], [0, FVD], [1, 2]], _CWH)
                Vv.tensor_tensor(out=_ap(mt2, [[FVD2, G], [2, FVD], [1, 2]]),
                                 in0=_ap(xyt, [[FVD2, G], [2, FVD], [1, 2]]),
                                 in1=wh_b, op=Alu.is_lt)
                Vv.tensor_tensor(out=mt1, in0=mt1, in1=mt2, op=Alu.mult)
                maskf = wk.tile([TP, FVD * G], F32, tag="maskf")
                mfv = _ap(maskf, [[FVD, G], [1, FVD]])
                Vv.tensor_tensor(out=mfv, in0=_ap(mt1, [[FVD2, G], [2, FVD]]),
                                 in1=_ap(mt1, [[FVD2, G], [2, FVD]], 1),
                                 op=Alu.mult)
                mzt = wk.tile([TP, FVD * G], F32, tag="mzt")
                mzv = _ap(mzt, [[FVD, G], [1, FVD]])
                Vv.tensor_scalar(out=mzv, in0=zsl, scalar1=float(EPS),
                                 scalar2=None, op0=Alu.is_gt)
                Vv.tensor_tensor(out=mfv, in0=mfv, in1=mzv, op=Alu.mult)

                mask_u8 = io.tile([TP, FVD * G], U8, tag="mask_u8")
                Vv.tensor_copy(out=mask_u8, in_=maskf)
                nc.sync.dma_start(
                    out=_dap(mask_d, r0 * FVD, FVD * G, [[1, FVD * G]]),
                    in_=mask_u8)

                # ---- LM / HM ----
                lmt = wk.tile([TP, FVD * G], F32, tag="lmt")
                Vv.tensor_scalar(out=lmt, in0=maskf, scalar1=-33.0,
                                 scalar2=None, op0=Alu.mult)
                d33_b = _ap(cn, [[0, G], [0, 6], [1, D]], _CD33)
                lmtv = _ap(lmt, [[FVD, G], [D, 6], [1, D]])
                Vv.tensor_tensor(out=lmtv, in0=lmtv, in1=d33_b, op=Alu.add)
                lm = wk.tile([TP, 6 * G], F32, tag="lm")
                Vv.tensor_reduce(out=lm, in_=lmtv, axis=mybir.AxisListType.X,
                                 op=Alu.min)
                hmt = wk.tile([TP, FVD * G], F32, tag="hmt")
                d1_b = _ap(cn, [[0, G], [0, 6], [1, D]], _CD1)
                hmtv = _ap(hmt, [[FVD, G], [D, 6], [1, D]])
                Vv.tensor_tensor(out=hmtv, in0=_ap(
                    maskf, [[FVD, G], [D, 6], [1, D]]), in1=d1_b, op=Alu.mult)
                hm = wk.tile([TP, 6 * G], F32, tag="hm")
                Vv.tensor_reduce(out=hm, in_=hmtv, axis=mybir.AxisListType.X,
                                 op=Alu.max)

                # ---- exact boundary counts ----
                s8 = big.tile([TP, FB * G], U8, tag="s8")
                Vv.tensor_scalar(out=s8, in0=As, scalar1=0.0, scalar2=None,
                                 op0=Alu.is_ge)
                x1 = big.tile([TP, FB * G], F32, tag="x1")
                Vv.reciprocal(out=x1, in_=As)                       # rA
                x2 = big.tile([TP, FB * G], F32, tag="x2")
                Vv.tensor_tensor(out=x2, in0=Bs, in1=x1, op=Alu.mult)  # t
                x3 = big.tile([TP, FB * G], F32, tag="x3")
                Vv.tensor_scalar(out=x3, in0=x2, scalar1=float(C1),
                                 scalar2=float(C2), op0=Alu.mult,
                                 op1=Alu.add)                        # w
                x1 = big.tile([TP, FB * G], F32, tag="x1")
                Vv.tensor_scalar(out=x1, in0=x3, scalar1=0.0, scalar2=1e6,
                                 op0=Alu.max, op1=Alu.min)           # wc
                x2 = big.tile([TP, FB * G], F32, tag="x2")
                Sc.activation(out=x2, in_=x1,
                              func=mybir.ActivationFunctionType.Sqrt)  # q
                x1 = big.tile([TP, FB * G], F32, tag="x1")
                Vv.tensor_scalar(out=x1, in0=x2, scalar1=1.0, scalar2=0.5,
                                 op0=Alu.subtract, op1=Alu.mult)     # hp
                xi = big.tile([TP, FB * G], I32, tag="xi")
                Vv.tensor_copy(out=xi, in_=x1)                       # h i32
                x2 = big.tile([TP, FB * G], F32, tag="x2")
                Vv.tensor_copy(out=x2, in_=xi)                       # h f32
                x1 = big.tile([TP, FB * G], F32, tag="x1")
                Vv.tensor_scalar(out=x1, in0=x2, scalar1=2.0, scalar2=1.0,
                                 op0=Alu.mult, op1=Alu.add)          # 2h+1
                x4 = big.tile([TP, FB * G], F32, tag="x4")
                Vv.tensor_tensor(out=x4, in0=x1, in1=x1, op=Alu.mult)  # bks
                x1 = big.tile([TP, FB * G], F32, tag="x1")
                Vv.tensor_tensor(out=x1, in0=x3, in1=x4, op=Alu.is_ge)  # cge
                x3 = big.tile([TP, FB * G], F32, tag="x3")
                Vv.tensor_tensor(out=x3, in0=x2, in1=x1, op=Alu.add)  # cnt
                x2 = big.tile([TP, FB * G], F32, tag="x2")
                nc.gpsimd.memset(x2[:, :], 0.0)
                Vv.copy_predicated(out=x2, mask=s8, data=x3)         # lo
                c32_b = _ap(cn, [[0, FB * G]], _C32)
                Vv.copy_predicated(out=x3, mask=s8, data=c32_b)      # hi

                # ---- combine ----
                lof = wk.tile([TP, FP * G], F32, tag="lof")
                Vv.tensor_reduce(
                    out=lof, in_=_ap(x2, [[FB, G], [1, FP], [FP, 4]]),
                    axis=mybir.AxisListType.X, op=Alu.max)
                lm_b = _ap(lm, [[6, G], [1, 6], [0, P]])
                lofv = _ap(lof, [[FP, G], [P, 6], [1, P]])
                Vv.tensor_tensor(out=lofv, in0=lofv, in1=lm_b, op=Alu.max)
                hif = wk.tile([TP, FP * G], F32, tag="hif")
                Vv.tensor_reduce(
                    out=hif, in_=_ap(x3, [[FB, G], [1, FP], [FP, 4]]),
                    axis=mybir.AxisListType.X, op=Alu.min)
                hm_b = _ap(hm, [[6, G], [1, 6], [0, P]])
                hifv = _ap(hif, [[FP, G], [P, 6], [1, P]])
                Vv.tensor_tensor(out=hifv, in0=hifv, in1=hm_b, op=Alu.min)

                corr_u8 = io.tile([TP, FP * G], U8, tag="corr_u8")
                Vv.tensor_tensor(out=corr_u8, in0=lof, in1=hif, op=Alu.is_lt)
                nc.sync.dma_start(
                    out=_dap(corr_d, r0 * FP, FP * G, [[1, FP * G]]),
                    in_=corr_u8)
                nc.sync.dma_start(
                    out=_dap(xy_d, r0 * FVD2, FVD2 * G, [[1, FVD2 * G]]),
                    in_=xyt)

    nc.finalize()
    return nc


def _host_tables(trans_mats, rois_pad):
    tm = np.asarray(trans_mats, dtype=np.float32)
    rois = np.asarray(rois_pad, dtype=np.float32)

    # wtr[g, i*24 + v*4 + j] = tm[v, g, i, j]  (i < 3)
    wtr = np.ascontiguousarray(
        tm[:, :, 0:3, :].transpose(1, 2, 0, 3)).reshape(6, 72)

    # f rows: [0:6]=onehot, [6:12]=onehot*x, [12:18]=onehot*y
    Wc = np.zeros((18, V, 3), np.float32)
    for blk, ct in enumerate((2, 0, 1)):
        Wc[blk * 6:(blk + 1) * 6] = tm[:, :, 0:3, ct].transpose(1, 0, 2)
    Wb = tm[:, :, 0:3, 3].transpose(1, 0, 2)  # [g, v, i]

    rhsA = np.zeros((18, 4, V, P), np.float32)
    rhsB = np.zeros((18, 4, V, P), np.float32)
    specs = [(0, True, 0), (2, False, 0), (1, True, 1), (3, False, 1)]
    for bnd, (ci, gt, ai) in enumerate(specs):
        c = rois[:, :, ci]                                   # [V, P]
        for v in range(V):
            if gt:
                rhsA[:, bnd, v, :] = (Wc[:, v, ai][:, None]
                                      - c[v][None, :] * Wc[:, v, 2][:, None])
                rhsB[0:6, bnd, v, :] = (c[v][None, :] * Wb[:, v, 2][:, None]
                                        - Wb[:, v, ai][:, None])
            else:
                rhsA[:, bnd, v, :] = (c[v][None, :] * Wc[:, v, 2][:, None]
                                      - Wc[:, v, ai][:, None])
                rhsB[0:6, bnd, v, :] = (Wb[:, v, ai][:, None]
                                        - c[v][None, :] * Wb[:, v, 2][:, None])
    rhsA = rhsA.reshape(18, FB).astype(np.float32)
    rhsB = rhsB.reshape(18, FB).astype(np.float32)

    idxf = np.arange(D, dtype=np.float32)
    dep = (S32 + BIN * idxf * (idxf + 1)).astype(np.float32)
    row = np.zeros(CW, np.float32)
    row[_CG:_CG + 6] = np.arange(6, dtype=np.float32)
    row[_CD:_CD + D] = dep
    row[_CD33:_CD33 + D] = idxf + 33.0
    row[_CD1:_CD1 + D] = idxf + 1.0
    row[_CWH] = W2D
    row[_CWH + 1] = H2D
    row[_C32] = 32.0
    consts = np.tile(row[None, :], (128, 1)).astype(np.float32)
    eye = np.eye(128, dtype=np.float32)
    return wtr, rhsA, rhsB, consts, eye


def kernel(points, trans_mats, rois_pad):
    points = np.asarray(points, dtype=np.float32)
    reps = int(_CACHE.get("reps", 1))
    key = ("nc", reps)
    if key not in _CACHE:
        _CACHE[key] = _build_nc(reps)
    nc = _CACHE[key]

    wtr, rhsA, rhsB, consts, eye = _host_tables(trans_mats, rois_pad)

    in_maps = []
    for c in range(NCORES):
        shard = np.zeros((NLP, 3), np.float32)
        shard[:NL] = points[c * NL:(c + 1) * NL]
        in_maps.append({"pts": shard, "wtr": wtr, "rhsA": rhsA,
                        "rhsB": rhsB, "consts": consts, "eye": eye})

    res = run_bass_kernel_spmd(nc, in_maps, core_ids=list(range(NCORES)))
    _CACHE["last"] = res
    outs = res.results

    xy = np.concatenate([r["xy"][:NL] for r in outs], axis=0)
    mask = np.concatenate([r["masko"][:NL] for r in outs], axis=0)
    corr = np.concatenate([r["corro"][:NL] for r in outs], axis=0)
    xy = xy.reshape(N, V, D, 2)
    mask = mask.reshape(N, V, D).astype(np.bool_)
    corr = corr.reshape(N, V, P).astype(np.bool_)
    return xy, mask, corr


# revision 11
# speedup vs baseline: 4.8255x; 2.5037x over previous
"""Trainium2 Bass kernel for nn_BoxCorrelation (epipolar point/box correlation).

Strategy
--------
Shard the N=20000 points across 8 NeuronCores (2500 each, padded to 2560).
Per tile, 128 partitions each hold G consecutive points; the free dimension
carries the per-point (view, box-boundary, depth) structure.

Host prep (O(N), input marshaling): per-point gathered view transforms
Mg = trans_mats[:, img_id] and the epipolar-line coefficient features; all
O(N*V*D) projection work and O(N*V*P) box-correlation work runs on device.

Device per tile:
  1. projection P[(pt,c,v,d)] = Mg . [x*d, y*d, d, 1] in reference op order,
     xy = P/max(z,eps), validity mask, contiguous valid-depth interval
     [LM, HM)  (monotonicity of the epipolar curve in depth).
  2. per-(boundary, view, box) half-line coefficients A, B from host
     features; each boundary's exact LID-grid index count is recovered with
     an approximate ACT sqrt plus an exact f32 compare (w >= (2h+1)^2),
     bit-identical to the reference comparisons.
  3. corr = (max of interval lower bounds) < (min of upper bounds).
"""

import numpy as np

import concourse.bass as bass
import concourse.bacc as bacc
import concourse.tile as tile
from concourse import mybir
from concourse.bass_utils import run_bass_kernel_spmd

# problem dims (hardcoded per spec)
N, V, P, D = 20000, 6, 64, 32
W2D, H2D = 1600.0, 928.0
NCORES = 8
NL = N // NCORES            # 2500 points per core
G = 4                       # points per partition
TP = 128                    # partitions
PTS_T = TP * G              # points per tile
NT = (NL + PTS_T - 1) // PTS_T
NLP = NT * PTS_T            # padded points per core

FB = 4 * V * P              # 1536 per point: (boundary, view, box)
FVD = V * D                 # 192
FVD2 = 2 * FVD              # 384
FP = V * P                  # 384
MGW = 72 * G + 2 * G        # mg block + x,y per point
ACW = 96                    # acoef block per point: (bnd, {u,w2,ub,w2b}, v)
MCW = FVD + FP              # mask + corr combined u8 row

S32 = np.float32(0.5)
BIN = np.float32(np.float32(70.0 - 0.5) / np.float32(D * (D + 1)))
INVBIN = np.float32(np.float32(1.0) / BIN)
C1 = np.float32(np.float32(4.0) * INVBIN)
C2 = np.float32(np.float32(1.0) - np.float32(4.0) * S32 * INVBIN)
EPS = np.float32(1e-5)

F32 = mybir.dt.float32
I32 = mybir.dt.int32
U8 = mybir.dt.uint8
Alu = mybir.AluOpType

# consts columns
_CD = 0            # 32: depth values
_CD33 = 32         # 32: d + 33
_CD1 = 64          # 32: d + 1
_CWH = 96          # 2: [W, H]
_C32 = 98          # 1: 32.0
CW = 100

_CACHE = {}

PADROW = np.zeros(ACW, np.float32)
for _bnd in range(4):
    PADROW[_bnd * 24 + 0:_bnd * 24 + 6] = 1.0   # u=1 -> A=1 on pad points



def _ap(base, pattern, off=0):
    """AP with base's partition dim + custom free [step, count] pattern."""
    return bass.AP(tensor=base.tensor, offset=base.offset + off,
                   ap=[base.ap[0]] + [list(p) for p in pattern])


def _dap(handle, poff, pstep, free):
    """DRAM AP: partition dim [pstep, 128] at element offset poff."""
    base = handle[:, :]
    return bass.AP(tensor=base.tensor, offset=poff,
                   ap=[[pstep, TP]] + [list(p) for p in free])


def _build_nc(reps=1):
    nc = bacc.Bacc(None, target_bir_lowering=False)

    mgp_d = nc.dram_tensor("mgp", [NLP // G, MGW], F32, kind="ExternalInput")
    acf_d = nc.dram_tensor("acf", [NLP, ACW], F32, kind="ExternalInput")
    ctab_d = nc.dram_tensor("ctab", [128, FB], F32, kind="ExternalInput")
    der_d = nc.dram_tensor("der", [128, 96 * G], F32, kind="ExternalInput")
    consts_d = nc.dram_tensor("consts", [128, CW], F32, kind="ExternalInput")

    xy_d = nc.dram_tensor("xy", [NLP, FVD2], F32, kind="ExternalOutput")
    mco_d = nc.dram_tensor("mco", [NLP, MCW], U8, kind="ExternalOutput")

    Vv = nc.vector
    Sc = nc.scalar
    BX = TP, FB * G

    with tile.TileContext(nc) as tc:
        with (
            tc.tile_pool(name="cst", bufs=1) as cst,
            tc.tile_pool(name="io", bufs=2) as io,
            tc.tile_pool(name="wk", bufs=1) as wk,
            tc.tile_pool(name="big", bufs=1) as big,
        ):
            cn = cst.tile([128, CW], F32)
            ctab = cst.tile([128, FB], F32)
            der = cst.tile([128, 96 * G], F32)
            nc.sync.dma_start(out=cn, in_=consts_d[:, :])
            nc.sync.dma_start(out=ctab, in_=ctab_d[:, :])
            nc.sync.dma_start(out=der, in_=der_d[:, :])

            for rep in range(reps):
             for it in range(NT):
                r0 = it * PTS_T       # first point row of this tile
                mgp = io.tile([TP, MGW], F32, tag="mgp")
                nc.sync.dma_start(
                    out=mgp, in_=_dap(mgp_d, r0 // G * MGW, MGW, [[1, MGW]]))
                acf = io.tile([TP, ACW * G], F32, tag="acf")
                nc.sync.dma_start(
                    out=acf,
                    in_=_dap(acf_d, r0 * ACW, ACW * G, [[1, ACW * G]]))

                # ---- per-term inputs (pt, c, d): x*d, y*d (dep is const) ----
                dep_b3 = _ap(cn, [[0, G], [0, 3], [1, D]], _CD)
                xdr = wk.tile([TP, 96 * G], F32, tag="xdr")
                Vv.tensor_tensor(out=_ap(xdr, [[96, G], [32, 3], [1, D]]),
                                 in0=_ap(mgp, [[2, G], [0, 3], [0, D]], 72 * G),
                                 in1=dep_b3, op=Alu.mult)
                ydr = wk.tile([TP, 96 * G], F32, tag="ydr")
                Vv.tensor_tensor(out=_ap(ydr, [[96, G], [32, 3], [1, D]]),
                                 in0=_ap(mgp, [[2, G], [0, 3], [0, D]],
                                         72 * G + 1),
                                 in1=dep_b3, op=Alu.mult)

                # ---- projection P[(pt, c, v, d)] = Mg . [x*d, y*d, d, 1] ----
                pall = big.tile([TP, 576 * G], F32, tag="bigA")
                ptmp = big.tile([TP, 576 * G], F32, tag="bigB")
                pa = _ap(pall, [[192, 3 * G], [32, 6], [1, D]])
                pb = _ap(ptmp, [[192, 3 * G], [32, 6], [1, D]])

                def mg_b(j):
                    return _ap(mgp, [[24, 3 * G], [4, 6], [0, D]], j)

                def vec_b(t, rep3=(32, 3)):
                    return _ap(t, [[32, 3 * G], [0, 6], [1, D]])

                Vv.tensor_tensor(out=pa, in0=vec_b(xdr), in1=mg_b(0),
                                 op=Alu.mult)
                Vv.tensor_tensor(out=pb, in0=vec_b(ydr), in1=mg_b(1),
                                 op=Alu.mult)
                Vv.tensor_tensor(out=pa, in0=pa, in1=pb, op=Alu.add)
                Vv.tensor_tensor(out=pb, in0=vec_b(der), in1=mg_b(2),
                                 op=Alu.mult)
                Vv.tensor_tensor(out=pa, in0=pa, in1=pb, op=Alu.add)
                Vv.tensor_tensor(out=pa, in0=pa, in1=mg_b(3), op=Alu.add)

                # ---- xy, mask ----
                zsl = _ap(pall, [[576, G], [1, FVD]], 384)
                zc = wk.tile([TP, FVD * G], F32, tag="zc")
                zcv = _ap(zc, [[FVD, G], [1, FVD]])
                Vv.tensor_scalar(out=zcv, in0=zsl, scalar1=float(EPS),
                                 scalar2=None, op0=Alu.max)
                rz = wk.tile([TP, FVD * G], F32, tag="rz")
                rzv = _ap(rz, [[FVD, G], [1, FVD]])
                Vv.reciprocal(out=rzv, in_=zcv)

                xyt = io.tile([TP, FVD2 * G], F32, tag="xyt")
                Vv.tensor_tensor(out=_ap(xyt, [[FVD2, G], [2, FVD]]),
                                 in0=_ap(pall, [[576, G], [1, FVD]]),
                                 in1=rzv, op=Alu.mult)
                Vv.tensor_tensor(out=_ap(xyt, [[FVD2, G], [2, FVD]], 1),
                                 in0=_ap(pall, [[576, G], [1, FVD]], 192),
                                 in1=rzv, op=Alu.mult)

                mzt = wk.tile([TP, FVD * G], F32, tag="mzt")
                mzv = _ap(mzt, [[FVD, G], [1, FVD]])
                Vv.tensor_scalar(out=mzv, in0=zsl, scalar1=float(EPS),
                                 scalar2=None, op0=Alu.is_gt)

                mt1 = big.tile([TP, FVD2 * G], F32, tag="bigC")
                mt2 = big.tile([TP, FVD2 * G], F32, tag="bigD")
                Vv.tensor_scalar(out=mt1, in0=xyt, scalar1=0.0, scalar2=None,
                                 op0=Alu.is_ge)
                wh_b = _ap(cn, [[0, G], [0, FVD], [1, 2]], _CWH)
                Vv.tensor_tensor(out=_ap(mt2, [[FVD2, G], [2, FVD], [1, 2]]),
                                 in0=_ap(xyt, [[FVD2, G], [2, FVD], [1, 2]]),
                                 in1=wh_b, op=Alu.is_lt)
                Vv.tensor_tensor(out=mt1, in0=mt1, in1=mt2, op=Alu.mult)
                maskf = wk.tile([TP, FVD * G], F32, tag="maskf")
                mfv = _ap(maskf, [[FVD, G], [1, FVD]])
                Vv.tensor_tensor(out=mfv, in0=_ap(mt1, [[FVD2, G], [2, FVD]]),
                                 in1=_ap(mt1, [[FVD2, G], [2, FVD]], 1),
                                 op=Alu.mult)
                Vv.tensor_tensor(out=mfv, in0=mfv, in1=mzv, op=Alu.mult)

                mco = io.tile([TP, MCW * G], U8, tag="mco")
                Vv.tensor_copy(out=_ap(mco, [[MCW, G], [1, FVD]]), in_=mfv)

                # ---- LM / HM ----
                lmt = wk.tile([TP, FVD * G], F32, tag="lmt")
                Vv.tensor_scalar(out=lmt, in0=maskf, scalar1=-33.0,
                                 scalar2=None, op0=Alu.mult)
                d33_b = _ap(cn, [[0, G], [0, 6], [1, D]], _CD33)
                lmtv = _ap(lmt, [[FVD, G], [D, 6], [1, D]])
                Vv.tensor_tensor(out=lmtv, in0=lmtv, in1=d33_b, op=Alu.add)
                lm = wk.tile([TP, 6 * G], F32, tag="lm")
                Vv.tensor_reduce(out=lm, in_=lmtv, axis=mybir.AxisListType.X,
                                 op=Alu.min)
                hmt = wk.tile([TP, FVD * G], F32, tag="hmt")
                d1_b = _ap(cn, [[0, G], [0, 6], [1, D]], _CD1)
                hmtv = _ap(hmt, [[FVD, G], [D, 6], [1, D]])
                Vv.tensor_tensor(out=hmtv, in0=_ap(
                    maskf, [[FVD, G], [D, 6], [1, D]]), in1=d1_b, op=Alu.mult)
                hm = wk.tile([TP, 6 * G], F32, tag="hm")
                Vv.tensor_reduce(out=hm, in_=hmtv, axis=mybir.AxisListType.X,
                                 op=Alu.max)

                # ---- A, B line coefficients on DVE ----
                # acf block (pt, bnd, {u, w2, ub, w2b}, v):
                #   A = u + c*w2,  B = ub + c*w2b   (c = ctab boundary value)
                ctab_b = _ap(ctab, [[0, G], [1, FB]])

                def acf_b(c4):
                    return _ap(acf, [[24, 4 * G], [1, 6], [0, P]], 6 * c4)

                t1 = big.tile([*BX], F32, tag="bigA")
                Vv.tensor_tensor(out=t1, in0=ctab_b, in1=acf_b(1),
                                 op=Alu.mult)
                As = big.tile([*BX], F32, tag="bigB")
                Vv.tensor_tensor(out=As, in0=t1, in1=acf_b(0), op=Alu.add)
                t2 = big.tile([*BX], F32, tag="bigA")
                Vv.tensor_tensor(out=t2, in0=ctab_b, in1=acf_b(3),
                                 op=Alu.mult)
                Bs = big.tile([*BX], F32, tag="bigC")
                Vv.tensor_tensor(out=Bs, in0=t2, in1=acf_b(2), op=Alu.add)

                # ---- exact boundary counts ----
                s8 = big.tile([*BX], U8, tag="s8")
                Vv.tensor_scalar(out=s8, in0=As, scalar1=0.0, scalar2=None,
                                 op0=Alu.is_ge)
                rA = big.tile([*BX], F32, tag="bigD")
                Vv.reciprocal(out=rA, in_=As)
                tb = big.tile([*BX], F32, tag="bigA")
                Vv.tensor_tensor(out=tb, in0=Bs, in1=rA, op=Alu.mult)
                wv = big.tile([*BX], F32, tag="bigC")
                Vv.tensor_scalar(out=wv, in0=tb, scalar1=float(C1),
                                 scalar2=float(C2), op0=Alu.mult, op1=Alu.add)
                wc = big.tile([*BX], F32, tag="bigD")
                Vv.tensor_scalar(out=wc, in0=wv, scalar1=0.0, scalar2=1e6,
                                 op0=Alu.max, op1=Alu.min)
                q = big.tile([*BX], F32, tag="bigB")
                Sc.activation(out=q, in_=wc,
                              func=mybir.ActivationFunctionType.Sqrt)
                hp = big.tile([*BX], F32, tag="bigD")
                Vv.tensor_scalar(out=hp, in0=q, scalar1=1.0, scalar2=0.5,
                                 op0=Alu.subtract, op1=Alu.mult)
                h32 = big.tile([*BX], I32, tag="bigA")
                Vv.tensor_copy(out=h32, in_=hp)
                hf = big.tile([*BX], F32, tag="bigB")
                Vv.tensor_copy(out=hf, in_=h32)
                oh2 = big.tile([*BX], F32, tag="bigD")
                Vv.tensor_scalar(out=oh2, in0=hf, scalar1=2.0, scalar2=1.0,
                                 op0=Alu.mult, op1=Alu.add)
                bks = big.tile([*BX], F32, tag="bigA")
                Vv.tensor_tensor(out=bks, in0=oh2, in1=oh2, op=Alu.mult)
                cge = big.tile([*BX], F32, tag="bigD")
                Vv.tensor_tensor(out=cge, in0=wv, in1=bks, op=Alu.is_ge)
                cnt = big.tile([*BX], F32, tag="bigC")
                Vv.tensor_tensor(out=cnt, in0=hf, in1=cge, op=Alu.add)
                lo = big.tile([*BX], F32, tag="bigA")
                Vv.memset(lo[:, :], 0.0)
                Vv.copy_predicated(out=lo, mask=s8, data=cnt)
                c32_b = _ap(cn, [[0, FB * G]], _C32)
                Vv.copy_predicated(out=cnt, mask=s8, data=c32_b)

                # ---- combine ----
                lof = wk.tile([TP, FP * G], F32, tag="lof")
                Vv.tensor_reduce(
                    out=lof, in_=_ap(lo, [[FB, G], [1, FP], [FP, 4]]),
                    axis=mybir.AxisListType.X, op=Alu.max)
                lm_b = _ap(lm, [[6, G], [1, 6], [0, P]])
                lofv = _ap(lof, [[FP, G], [P, 6], [1, P]])
                Vv.tensor_tensor(out=lofv, in0=lofv, in1=lm_b, op=Alu.max)
                hif = wk.tile([TP, FP * G], F32, tag="hif")
                Vv.tensor_reduce(
                    out=hif, in_=_ap(cnt, [[FB, G], [1, FP], [FP, 4]]),
                    axis=mybir.AxisListType.X, op=Alu.min)
                hm_b = _ap(hm, [[6, G], [1, 6], [0, P]])
                hifv = _ap(hif, [[FP, G], [P, 6], [1, P]])
                Vv.tensor_tensor(out=hifv, in0=hifv, in1=hm_b, op=Alu.min)

                Vv.tensor_tensor(out=_ap(mco, [[MCW, G], [1, FP]], FVD),
                                 in0=lof, in1=hif, op=Alu.is_lt)
                nc.sync.dma_start(
                    out=_dap(mco_d, r0 * MCW, MCW * G, [[1, MCW * G]]),
                    in_=mco)
                nc.sync.dma_start(
                    out=_dap(xy_d, r0 * FVD2, FVD2 * G, [[1, FVD2 * G]]),
                    in_=xyt)

    nc.finalize()
    return nc


def _host_tables(trans_mats, rois_pad):
    tm = np.asarray(trans_mats, dtype=np.float32)
    rois = np.asarray(rois_pad, dtype=np.float32)

    # ctab[(bnd, v, p)] = boundary value
    specs = [(0, True, 0), (2, False, 0), (1, True, 1), (3, False, 1)]
    crow = np.zeros((4, V, P), np.float32)
    for bnd, (ci, _, _) in enumerate(specs):
        crow[bnd] = rois[:, :, ci]
    ctab = np.tile(crow.reshape(1, FB), (128, 1)).astype(np.float32)

    idxf = np.arange(D, dtype=np.float32)
    dep = (S32 + BIN * idxf * (idxf + 1)).astype(np.float32)
    der = np.tile(dep[None, :], (128, 3 * G)).astype(np.float32)

    row = np.zeros(CW, np.float32)
    row[_CD:_CD + D] = dep
    row[_CD33:_CD33 + D] = idxf + 33.0
    row[_CD1:_CD1 + D] = idxf + 1.0
    row[_CWH] = W2D
    row[_CWH + 1] = H2D
    row[_C32] = 32.0
    consts = np.tile(row[None, :], (128, 1)).astype(np.float32)
    return ctab, der, consts


def _host_points(points, trans_mats):
    """Per-point gathered transforms + line-coef features, padded/tiled."""
    tm = np.asarray(trans_mats, dtype=np.float32)
    pts = np.asarray(points, dtype=np.float32)
    n = pts.shape[0]
    g = pts[:, 0].astype(np.int32)
    x = pts[:, 1]
    y = pts[:, 2]

    M = tm[:, g]                               # [V, n, 4, 4]
    # mg[(i, v, j)] per point
    mg = np.ascontiguousarray(
        M[:, :, 0:3, :].transpose(1, 2, 0, 3)).reshape(n, 72)

    # a_i = M[i,0]*x + M[i,1]*y + M[i,2],  b_i = M[i,3]   [n, V, 3]
    a = (M[:, :, 0:3, 0] * x[None, :, None]
         + M[:, :, 0:3, 1] * y[None, :, None]
         + M[:, :, 0:3, 2]).astype(np.float32).transpose(1, 0, 2)
    b = M[:, :, 0:3, 3].transpose(1, 0, 2)     # [n, V, 3]

    # acf[(bnd, {u, w2, ub, w2b}, v)]:
    #   gt (sgn=+1): A = a_i - c*a2, B = c*b2 - b_i
    #   lt (sgn=-1): A = c*a2 - a_i, B = b_i - c*b2
    # A = u + c*w2 with u = sgn*a_i, w2 = -sgn*a2
    # B = ub + c*w2b with ub = -sgn*b_i, w2b = sgn*b2
    acf = np.zeros((n, 4, 4, V), np.float32)
    specs = [(0, True, 0), (2, False, 0), (1, True, 1), (3, False, 1)]
    for bnd, (_, gt, ai) in enumerate(specs):
        sgn = np.float32(1.0) if gt else np.float32(-1.0)
        acf[:, bnd, 0] = sgn * a[:, :, ai]
        acf[:, bnd, 1] = -sgn * a[:, :, 2]
        acf[:, bnd, 2] = -sgn * b[:, :, ai]
        acf[:, bnd, 3] = sgn * b[:, :, 2]
    acf = acf.reshape(n, ACW)
    return mg, acf, x, y


def kernel(points, trans_mats, rois_pad):
    points = np.asarray(points, dtype=np.float32)
    reps = int(_CACHE.get("reps", 1))
    key = ("nc", reps)
    if key not in _CACHE:
        _CACHE[key] = _build_nc(reps)
    nc = _CACHE[key]

    ctab, der, consts = _host_tables(trans_mats, rois_pad)
    mg, acf, x, y = _host_points(points, trans_mats)

    in_maps = []
    for c in range(NCORES):
        sl = slice(c * NL, (c + 1) * NL)
        mgp = np.zeros((NLP // G, MGW), np.float32)
        acfp = np.zeros((NLP, ACW), np.float32)
        mgc = np.zeros((NLP, 72), np.float32)
        mgc[:NL] = mg[sl]
        xc = np.zeros(NLP, np.float32)
        yc = np.zeros(NLP, np.float32)
        xc[:NL] = x[sl]
        yc[:NL] = y[sl]
        acfp[:NL] = acf[sl]
        acfp[NL:] = PADROW
        # partition row holds G consecutive points
        mgp[:, :72 * G] = mgc.reshape(NLP // G, 72 * G)
        mgp[:, 72 * G + 0::2] = xc.reshape(NLP // G, G)
        mgp[:, 72 * G + 1::2] = yc.reshape(NLP // G, G)
        in_maps.append({"mgp": mgp, "acf": acfp, "ctab": ctab,
                        "der": der, "consts": consts})

    res = run_bass_kernel_spmd(nc, in_maps, core_ids=list(range(NCORES)))
    _CACHE["last"] = res
    outs = res.results

    xy = np.concatenate([r["xy"][:NL] for r in outs], axis=0)
    mco = np.concatenate([r["mco"][:NL] for r in outs], axis=0)
    xy = xy.reshape(N, V, D, 2)
    mask = mco[:, :FVD].reshape(N, V, D).astype(np.bool_)
    corr = mco[:, FVD:].reshape(N, V, P).astype(np.bool_)
    return xy, mask, corr


# revision 12
# speedup vs baseline: 6.9310x; 1.4363x over previous
"""Trainium2 Bass kernel for nn_BoxCorrelation (epipolar point/box correlation).

Strategy
--------
Shard the N=20000 points across 8 NeuronCores (2500 each, padded to 2560).
Per tile, 128 partitions each hold G consecutive points; the free dimension
carries the per-point (view, box-boundary, depth) structure.

Host prep (O(N), input marshaling): per-point gathered view transforms
Mg = trans_mats[:, img_id] and the epipolar-line coefficient features; all
O(N*V*D) projection work and O(N*V*P) box-correlation work runs on device.

Device per tile:
  1. projection P[(pt,c,v,d)] = Mg . [x*d, y*d, d, 1] in reference op order,
     xy = P/max(z,eps), validity mask, contiguous valid-depth interval
     [LM, HM)  (monotonicity of the epipolar curve in depth).
  2. per-(boundary, view, box) half-line coefficients A, B from host
     features; each boundary's exact LID-grid index count is recovered with
     an approximate ACT sqrt plus an exact f32 compare (w >= (2h+1)^2),
     bit-identical to the reference comparisons.
  3. corr = (max of interval lower bounds) < (min of upper bounds).
"""

import numpy as np

import concourse.bass as bass
import concourse.bacc as bacc
import concourse.tile as tile
from concourse import mybir
from concourse.bass_utils import run_bass_kernel_spmd

# problem dims (hardcoded per spec)
N, V, P, D = 20000, 6, 64, 32
W2D, H2D = 1600.0, 928.0
NCORES = 8
NL = N // NCORES            # 2500 points per core
G = 4                       # points per partition
TP = 128                    # partitions
PTS_T = TP * G              # points per tile
NT = (NL + PTS_T - 1) // PTS_T
NLP = NT * PTS_T            # padded points per core

FB = 4 * V * P              # 1536 per point: (boundary, view, box)
FVD = V * D                 # 192
FVD2 = 2 * FVD              # 384
FP = V * P                  # 384
INW = 132 * G               # input row: a(18) | b(18) | acf(96) per point
ACW = 96                    # acoef block per point: (bnd, {u,w2,ub,w2b}, v)
MCW = FVD + FP              # mask + corr combined u8 row

S32 = np.float32(0.5)
BIN = np.float32(np.float32(70.0 - 0.5) / np.float32(D * (D + 1)))
INVBIN = np.float32(np.float32(1.0) / BIN)
C1 = np.float32(np.float32(4.0) * INVBIN)
C2 = np.float32(np.float32(1.0) - np.float32(4.0) * S32 * INVBIN)
EPS = np.float32(1e-5)

F32 = mybir.dt.float32
I32 = mybir.dt.int32
U8 = mybir.dt.uint8
Alu = mybir.AluOpType

# consts columns
_CD = 0            # 32: depth values
_CD33 = 32         # 32: d + 33
_CD1 = 64          # 32: d + 1
_CWH = 96          # 2: [W, H]
_C32 = 98          # 1: 32.0
CW = 100

_CACHE = {}

PADROW = np.zeros(ACW, np.float32)
for _bnd in range(4):
    PADROW[_bnd * 24 + 0:_bnd * 24 + 6] = 1.0   # u=1 -> A=1 on pad points



def _ap(base, pattern, off=0):
    """AP with base's partition dim + custom free [step, count] pattern."""
    return bass.AP(tensor=base.tensor, offset=base.offset + off,
                   ap=[base.ap[0]] + [list(p) for p in pattern])


def _dap(handle, poff, pstep, free):
    """DRAM AP: partition dim [pstep, 128] at element offset poff."""
    base = handle[:, :]
    return bass.AP(tensor=base.tensor, offset=poff,
                   ap=[[pstep, TP]] + [list(p) for p in free])


def _build_nc(reps=1):
    nc = bacc.Bacc(None, target_bir_lowering=False)

    inr_d = nc.dram_tensor("inr", [NLP // G, INW], F32, kind="ExternalInput")
    ctab_d = nc.dram_tensor("ctab", [128, FB], F32, kind="ExternalInput")
    consts_d = nc.dram_tensor("consts", [128, CW], F32, kind="ExternalInput")

    xy_d = nc.dram_tensor("xy", [NLP, FVD2], F32, kind="ExternalOutput")
    mco_d = nc.dram_tensor("mco", [NLP, MCW], U8, kind="ExternalOutput")

    Vv = nc.vector
    Sc = nc.scalar
    BX = TP, FB * G

    with tile.TileContext(nc) as tc:
        with (
            tc.tile_pool(name="cst", bufs=1) as cst,
            tc.tile_pool(name="io", bufs=2) as io,
            tc.tile_pool(name="wk", bufs=1) as wk,
            tc.tile_pool(name="big", bufs=1) as big,
        ):
            cn = cst.tile([128, CW], F32)
            ctab = cst.tile([128, FB], F32)
            nc.sync.dma_start(out=cn, in_=consts_d[:, :])
            nc.sync.dma_start(out=ctab, in_=ctab_d[:, :])

            for rep in range(reps):
             for it in range(NT):
                r0 = it * PTS_T       # first point row of this tile
                inr = io.tile([TP, INW], F32, tag="inr")
                nc.sync.dma_start(
                    out=inr, in_=_dap(inr_d, r0 // G * INW, INW, [[1, INW]]))

                # ---- projection P[(pt, c, v, d)] = a*dep + b ----
                pall = big.tile([TP, 576 * G], F32, tag="bigA")
                pa = _ap(pall, [[192, 3 * G], [32, 6], [1, D]])
                dep_bb = _ap(cn, [[0, 3 * G], [0, 6], [1, D]], _CD)
                a_b = _ap(inr, [[6, 3 * G], [1, 6], [0, D]])
                b_b = _ap(inr, [[6, 3 * G], [1, 6], [0, D]], 18 * G)
                Vv.tensor_tensor(out=pa, in0=a_b, in1=dep_bb, op=Alu.mult)
                Vv.tensor_tensor(out=pa, in0=pa, in1=b_b, op=Alu.add)

                # ---- xy, mask ----
                zsl = _ap(pall, [[576, G], [1, FVD]], 384)
                zc = wk.tile([TP, FVD * G], F32, tag="zc")
                zcv = _ap(zc, [[FVD, G], [1, FVD]])
                Vv.tensor_scalar(out=zcv, in0=zsl, scalar1=float(EPS),
                                 scalar2=None, op0=Alu.max)
                rz = wk.tile([TP, FVD * G], F32, tag="rz")
                rzv = _ap(rz, [[FVD, G], [1, FVD]])
                Vv.reciprocal(out=rzv, in_=zcv)

                xyt = io.tile([TP, FVD2 * G], F32, tag="xyt")
                Vv.tensor_tensor(out=_ap(xyt, [[FVD2, G], [2, FVD]]),
                                 in0=_ap(pall, [[576, G], [1, FVD]]),
                                 in1=rzv, op=Alu.mult)
                Vv.tensor_tensor(out=_ap(xyt, [[FVD2, G], [2, FVD]], 1),
                                 in0=_ap(pall, [[576, G], [1, FVD]], 192),
                                 in1=rzv, op=Alu.mult)

                mzt = wk.tile([TP, FVD * G], F32, tag="mzt")
                mzv = _ap(mzt, [[FVD, G], [1, FVD]])
                Vv.tensor_scalar(out=mzv, in0=zsl, scalar1=float(EPS),
                                 scalar2=None, op0=Alu.is_gt)

                mt1 = big.tile([TP, FVD2 * G], F32, tag="bigC")
                mt2 = big.tile([TP, FVD2 * G], F32, tag="bigD")
                Vv.tensor_scalar(out=mt1, in0=xyt, scalar1=0.0, scalar2=None,
                                 op0=Alu.is_ge)
                wh_b = _ap(cn, [[0, G], [0, FVD], [1, 2]], _CWH)
                Vv.tensor_tensor(out=_ap(mt2, [[FVD2, G], [2, FVD], [1, 2]]),
                                 in0=_ap(xyt, [[FVD2, G], [2, FVD], [1, 2]]),
                                 in1=wh_b, op=Alu.is_lt)
                Vv.tensor_tensor(out=mt1, in0=mt1, in1=mt2, op=Alu.mult)
                maskf = wk.tile([TP, FVD * G], F32, tag="maskf")
                mfv = _ap(maskf, [[FVD, G], [1, FVD]])
                Vv.tensor_tensor(out=mfv, in0=_ap(mt1, [[FVD2, G], [2, FVD]]),
                                 in1=_ap(mt1, [[FVD2, G], [2, FVD]], 1),
                                 op=Alu.mult)
                Vv.tensor_tensor(out=mfv, in0=mfv, in1=mzv, op=Alu.mult)

                mco = io.tile([TP, MCW * G], U8, tag="mco")
                Vv.tensor_copy(out=_ap(mco, [[MCW, G], [1, FVD]]), in_=mfv)

                # ---- LM / HM ----
                lmt = wk.tile([TP, FVD * G], F32, tag="lmt")
                Vv.tensor_scalar(out=lmt, in0=maskf, scalar1=-33.0,
                                 scalar2=None, op0=Alu.mult)
                d33_b = _ap(cn, [[0, G], [0, 6], [1, D]], _CD33)
                lmtv = _ap(lmt, [[FVD, G], [D, 6], [1, D]])
                Vv.tensor_tensor(out=lmtv, in0=lmtv, in1=d33_b, op=Alu.add)
                lm = wk.tile([TP, 6 * G], F32, tag="lm")
                Vv.tensor_reduce(out=lm, in_=lmtv, axis=mybir.AxisListType.X,
                                 op=Alu.min)
                hmt = wk.tile([TP, FVD * G], F32, tag="hmt")
                d1_b = _ap(cn, [[0, G], [0, 6], [1, D]], _CD1)
                hmtv = _ap(hmt, [[FVD, G], [D, 6], [1, D]])
                Vv.tensor_tensor(out=hmtv, in0=_ap(
                    maskf, [[FVD, G], [D, 6], [1, D]]), in1=d1_b, op=Alu.mult)
                hm = wk.tile([TP, 6 * G], F32, tag="hm")
                Vv.tensor_reduce(out=hm, in_=hmtv, axis=mybir.AxisListType.X,
                                 op=Alu.max)

                # ---- A, B line coefficients on DVE ----
                # acf block (pt, bnd, {u, w2, ub, w2b}, v):
                #   A = u + c*w2,  B = ub + c*w2b   (c = ctab boundary value)
                ctab_b = _ap(ctab, [[0, G], [1, FB]])

                def acf_b(c4):
                    return _ap(inr, [[24, 4 * G], [1, 6], [0, P]],
                               36 * G + 6 * c4)

                t1 = big.tile([*BX], F32, tag="bigA")
                Vv.tensor_tensor(out=t1, in0=ctab_b, in1=acf_b(1),
                                 op=Alu.mult)
                As = big.tile([*BX], F32, tag="bigB")
                Vv.tensor_tensor(out=As, in0=t1, in1=acf_b(0), op=Alu.add)
                t2 = big.tile([*BX], F32, tag="bigA")
                Vv.tensor_tensor(out=t2, in0=ctab_b, in1=acf_b(3),
                                 op=Alu.mult)
                Bs = big.tile([*BX], F32, tag="bigC")
                Vv.tensor_tensor(out=Bs, in0=t2, in1=acf_b(2), op=Alu.add)

                # ---- exact boundary counts ----
                s8 = big.tile([*BX], U8, tag="s8")
                Vv.tensor_scalar(out=s8, in0=As, scalar1=0.0, scalar2=None,
                                 op0=Alu.is_ge)
                rA = big.tile([*BX], F32, tag="bigD")
                Vv.reciprocal(out=rA, in_=As)
                tb = big.tile([*BX], F32, tag="bigA")
                Vv.tensor_tensor(out=tb, in0=Bs, in1=rA, op=Alu.mult)
                wv = big.tile([*BX], F32, tag="bigC")
                Vv.tensor_scalar(out=wv, in0=tb, scalar1=float(C1),
                                 scalar2=float(C2), op0=Alu.mult, op1=Alu.add)
                wc = big.tile([*BX], F32, tag="bigD")
                Vv.tensor_scalar(out=wc, in0=wv, scalar1=0.0, scalar2=1e6,
                                 op0=Alu.max, op1=Alu.min)
                q = big.tile([*BX], F32, tag="bigB")
                Sc.activation(out=q, in_=wc,
                              func=mybir.ActivationFunctionType.Sqrt)
                hp = big.tile([*BX], F32, tag="bigD")
                Vv.tensor_scalar(out=hp, in0=q, scalar1=1.0, scalar2=0.5,
                                 op0=Alu.subtract, op1=Alu.mult)
                h32 = big.tile([*BX], I32, tag="bigA")
                Vv.tensor_copy(out=h32, in_=hp)
                hf = big.tile([*BX], F32, tag="bigB")
                Vv.tensor_copy(out=hf, in_=h32)
                oh2 = big.tile([*BX], F32, tag="bigD")
                Vv.tensor_scalar(out=oh2, in0=hf, scalar1=2.0, scalar2=1.0,
                                 op0=Alu.mult, op1=Alu.add)
                bks = big.tile([*BX], F32, tag="bigA")
                Vv.tensor_tensor(out=bks, in0=oh2, in1=oh2, op=Alu.mult)
                cge = big.tile([*BX], F32, tag="bigD")
                Vv.tensor_tensor(out=cge, in0=wv, in1=bks, op=Alu.is_ge)
                cnt = big.tile([*BX], F32, tag="bigC")
                Vv.tensor_tensor(out=cnt, in0=hf, in1=cge, op=Alu.add)
                lo = big.tile([*BX], F32, tag="bigA")
                Vv.memset(lo[:, :], 0.0)
                Vv.copy_predicated(out=lo, mask=s8, data=cnt)
                c32_b = _ap(cn, [[0, FB * G]], _C32)
                Vv.copy_predicated(out=cnt, mask=s8, data=c32_b)

                # ---- combine ----
                lof = wk.tile([TP, FP * G], F32, tag="lof")
                Vv.tensor_reduce(
                    out=lof, in_=_ap(lo, [[FB, G], [1, FP], [FP, 4]]),
                    axis=mybir.AxisListType.X, op=Alu.max)
                lm_b = _ap(lm, [[6, G], [1, 6], [0, P]])
                lofv = _ap(lof, [[FP, G], [P, 6], [1, P]])
                Vv.tensor_tensor(out=lofv, in0=lofv, in1=lm_b, op=Alu.max)
                hif = wk.tile([TP, FP * G], F32, tag="hif")
                Vv.tensor_reduce(
                    out=hif, in_=_ap(cnt, [[FB, G], [1, FP], [FP, 4]]),
                    axis=mybir.AxisListType.X, op=Alu.min)
                hm_b = _ap(hm, [[6, G], [1, 6], [0, P]])
                hifv = _ap(hif, [[FP, G], [P, 6], [1, P]])
                Vv.tensor_tensor(out=hifv, in0=hifv, in1=hm_b, op=Alu.min)

                Vv.tensor_tensor(out=_ap(mco, [[MCW, G], [1, FP]], FVD),
                                 in0=lof, in1=hif, op=Alu.is_lt)
                nc.sync.dma_start(
                    out=_dap(mco_d, r0 * MCW, MCW * G, [[1, MCW * G]]),
                    in_=mco)
                nc.sync.dma_start(
                    out=_dap(xy_d, r0 * FVD2, FVD2 * G, [[1, FVD2 * G]]),
                    in_=xyt)

    nc.finalize()
    return nc


def _host_tables(trans_mats, rois_pad):
    tm = np.asarray(trans_mats, dtype=np.float32)
    rois = np.asarray(rois_pad, dtype=np.float32)

    # ctab[(bnd, v, p)] = boundary value
    specs = [(0, True, 0), (2, False, 0), (1, True, 1), (3, False, 1)]
    crow = np.zeros((4, V, P), np.float32)
    for bnd, (ci, _, _) in enumerate(specs):
        crow[bnd] = rois[:, :, ci]
    ctab = np.tile(crow.reshape(1, FB), (128, 1)).astype(np.float32)

    idxf = np.arange(D, dtype=np.float32)
    dep = (S32 + BIN * idxf * (idxf + 1)).astype(np.float32)

    row = np.zeros(CW, np.float32)
    row[_CD:_CD + D] = dep
    row[_CD33:_CD33 + D] = idxf + 33.0
    row[_CD1:_CD1 + D] = idxf + 1.0
    row[_CWH] = W2D
    row[_CWH + 1] = H2D
    row[_C32] = 32.0
    consts = np.tile(row[None, :], (128, 1)).astype(np.float32)
    return ctab, consts


def _host_points(points, trans_mats):
    """Per-point gathered transforms + line-coef features, padded/tiled."""
    tm = np.asarray(trans_mats, dtype=np.float32)
    pts = np.asarray(points, dtype=np.float32)
    n = pts.shape[0]
    g = pts[:, 0].astype(np.int32)
    x = pts[:, 1]
    y = pts[:, 2]

    M = tm[:, g]                               # [V, n, 4, 4]

    # a_i = M[i,0]*x + M[i,1]*y + M[i,2],  b_i = M[i,3]   [n, V, 3]
    a = (M[:, :, 0:3, 0] * x[None, :, None]
         + M[:, :, 0:3, 1] * y[None, :, None]
         + M[:, :, 0:3, 2]).astype(np.float32).transpose(1, 0, 2)
    b = M[:, :, 0:3, 3].transpose(1, 0, 2)     # [n, V, 3]

    # acf[(bnd, {u, w2, ub, w2b}, v)]:
    #   gt (sgn=+1): A = a_i - c*a2, B = c*b2 - b_i
    #   lt (sgn=-1): A = c*a2 - a_i, B = b_i - c*b2
    # A = u + c*w2 with u = sgn*a_i, w2 = -sgn*a2
    # B = ub + c*w2b with ub = -sgn*b_i, w2b = sgn*b2
    acf = np.zeros((n, 4, 4, V), np.float32)
    specs = [(0, True, 0), (2, False, 0), (1, True, 1), (3, False, 1)]
    for bnd, (_, gt, ai) in enumerate(specs):
        sgn = np.float32(1.0) if gt else np.float32(-1.0)
        acf[:, bnd, 0] = sgn * a[:, :, ai]
        acf[:, bnd, 1] = -sgn * a[:, :, 2]
        acf[:, bnd, 2] = -sgn * b[:, :, ai]
        acf[:, bnd, 3] = sgn * b[:, :, 2]
    acf = acf.reshape(n, ACW)
    a18 = np.ascontiguousarray(a.transpose(0, 2, 1)).reshape(n, 18)
    b18 = np.ascontiguousarray(b.transpose(0, 2, 1)).reshape(n, 18)
    return a18, b18, acf


def kernel(points, trans_mats, rois_pad):
    points = np.asarray(points, dtype=np.float32)
    reps = int(_CACHE.get("reps", 1))
    key = ("nc", reps)
    if key not in _CACHE:
        _CACHE[key] = _build_nc(reps)
    nc = _CACHE[key]

    ctab, consts = _host_tables(trans_mats, rois_pad)
    a18, b18, acf = _host_points(points, trans_mats)

    in_maps = []
    for c in range(NCORES):
        sl = slice(c * NL, (c + 1) * NL)
        ac = np.zeros((NLP, 18), np.float32)
        bc = np.zeros((NLP, 18), np.float32)
        acfp = np.zeros((NLP, ACW), np.float32)
        ac[:NL] = a18[sl]
        bc[:NL] = b18[sl]
        acfp[:NL] = acf[sl]
        acfp[NL:] = PADROW
        inr = np.concatenate([ac.reshape(NLP // G, 18 * G),
                              bc.reshape(NLP // G, 18 * G),
                              acfp.reshape(NLP // G, ACW * G)], axis=1)
        in_maps.append({"inr": np.ascontiguousarray(inr), "ctab": ctab,
                        "consts": consts})

    res = run_bass_kernel_spmd(nc, in_maps, core_ids=list(range(NCORES)))
    _CACHE["last"] = res
    outs = res.results

    xy = np.concatenate([r["xy"][:NL] for r in outs], axis=0)
    mco = np.concatenate([r["mco"][:NL] for r in outs], axis=0)
    xy = xy.reshape(N, V, D, 2)
    mask = mco[:, :FVD].reshape(N, V, D).astype(np.bool_)
    corr = mco[:, FVD:].reshape(N, V, P).astype(np.bool_)
    return xy, mask, corr


# revision 13
# speedup vs baseline: 10.3028x; 1.4865x over previous
"""Trainium2 Bass kernel for nn_BoxCorrelation (epipolar point/box correlation).

Strategy
--------
Shard the N=20000 points across 8 NeuronCores (2500 each, padded to 2560).
Per tile, 128 partitions each hold G consecutive points; the free dimension
carries the per-point (view, box-boundary, depth) structure.

Host prep (O(N), input marshaling): per-point gathered view transforms
Mg = trans_mats[:, img_id] and the epipolar-line coefficient features; all
O(N*V*D) projection work and O(N*V*P) box-correlation work runs on device.

Device per tile:
  1. projection P[(pt,c,v,d)] = Mg . [x*d, y*d, d, 1] in reference op order,
     xy = P/max(z,eps), validity mask, contiguous valid-depth interval
     [LM, HM)  (monotonicity of the epipolar curve in depth).
  2. per-(boundary, view, box) half-line coefficients A, B from host
     features; each boundary's exact LID-grid index count is recovered with
     an approximate ACT sqrt plus an exact f32 compare (w >= (2h+1)^2),
     bit-identical to the reference comparisons.
  3. corr = (max of interval lower bounds) < (min of upper bounds).
"""

import numpy as np

import concourse.bass as bass
import concourse.bacc as bacc
import concourse.tile as tile
from concourse import mybir
from concourse.bass_utils import run_bass_kernel_spmd

# problem dims (hardcoded per spec)
N, V, P, D = 20000, 6, 64, 32
W2D, H2D = 1600.0, 928.0
NCORES = 8
NL = N // NCORES            # 2500 points per core
G = 5                       # points per partition
TP = 128                    # partitions
PTS_T = TP * G              # points per tile
NT = (NL + PTS_T - 1) // PTS_T
NLP = NT * PTS_T            # padded points per core

FB = 4 * V * P              # 1536 per point: (boundary, view, box)
FVD = V * D                 # 192
FVD2 = 2 * FVD              # 384
FP = V * P                  # 384
INW = 132 * G               # input row: a(18) | b(18) | acf(96) per point
ACW = 96                    # acoef block per point: (bnd, {u,w2,ub,w2b}, v)
MCW = FVD + FP              # mask + corr combined u8 row

S32 = np.float32(0.5)
BIN = np.float32(np.float32(70.0 - 0.5) / np.float32(D * (D + 1)))
INVBIN = np.float32(np.float32(1.0) / BIN)
C1 = np.float32(np.float32(4.0) * INVBIN)
C2 = np.float32(np.float32(1.0) - np.float32(4.0) * S32 * INVBIN)
EPS = np.float32(1e-5)

F32 = mybir.dt.float32
I32 = mybir.dt.int32
U8 = mybir.dt.uint8
Alu = mybir.AluOpType

# consts columns
_CD = 0            # 32: depth values
_CD33 = 32         # 32: d + 33
_CD1 = 64          # 32: d + 1
_CWH = 96          # 2: [W, H]
_C32 = 98          # 1: 32.0
CW = 100

_CACHE = {}

PADROW = np.zeros(ACW, np.float32)
for _bnd in range(4):
    PADROW[_bnd * 24 + 0:_bnd * 24 + 6] = 1.0   # u=1 -> A=1 on pad points



def _ap(base, pattern, off=0):
    """AP with base's partition dim + custom free [step, count] pattern."""
    return bass.AP(tensor=base.tensor, offset=base.offset + off,
                   ap=[base.ap[0]] + [list(p) for p in pattern])


def _dap(handle, poff, pstep, free):
    """DRAM AP: partition dim [pstep, 128] at element offset poff."""
    base = handle[:, :]
    return bass.AP(tensor=base.tensor, offset=poff,
                   ap=[[pstep, TP]] + [list(p) for p in free])


def _build_nc(reps=1):
    nc = bacc.Bacc(None, target_bir_lowering=False)

    inr_d = nc.dram_tensor("inr", [NLP // G, INW], F32, kind="ExternalInput")
    ctab_d = nc.dram_tensor("ctab", [128, FB], F32, kind="ExternalInput")
    consts_d = nc.dram_tensor("consts", [128, CW], F32, kind="ExternalInput")

    xy_d = nc.dram_tensor("xy", [NLP, FVD2], F32, kind="ExternalOutput")
    mco_d = nc.dram_tensor("mco", [NLP, MCW], U8, kind="ExternalOutput")

    Vv = nc.vector
    Sc = nc.scalar
    BX = TP, FB * G

    with tile.TileContext(nc) as tc:
        with (
            tc.tile_pool(name="cst", bufs=1) as cst,
            tc.tile_pool(name="io", bufs=2) as io,
            tc.tile_pool(name="wk", bufs=1) as wk,
            tc.tile_pool(name="big", bufs=1) as big,
        ):
            cn = cst.tile([128, CW], F32)
            ctab = cst.tile([128, FB], F32)
            nc.sync.dma_start(out=cn, in_=consts_d[:, :])
            nc.sync.dma_start(out=ctab, in_=ctab_d[:, :])

            for rep in range(reps):
             for it in range(NT):
                r0 = it * PTS_T       # first point row of this tile
                inr = io.tile([TP, INW], F32, tag="inr")
                nc.sync.dma_start(
                    out=inr, in_=_dap(inr_d, r0 // G * INW, INW, [[1, INW]]))

                # ---- projection P[(pt, c, v, d)] = a*dep + b ----
                pall = big.tile([TP, 576 * G], F32, tag="bigA")
                pa = _ap(pall, [[192, 3 * G], [32, 6], [1, D]])
                dep_bb = _ap(cn, [[0, 3 * G], [0, 6], [1, D]], _CD)
                a_b = _ap(inr, [[6, 3 * G], [1, 6], [0, D]])
                b_b = _ap(inr, [[6, 3 * G], [1, 6], [0, D]], 18 * G)
                Vv.tensor_tensor(out=pa, in0=a_b, in1=dep_bb, op=Alu.mult)
                Vv.tensor_tensor(out=pa, in0=pa, in1=b_b, op=Alu.add)

                # ---- xy, mask ----
                zsl = _ap(pall, [[576, G], [1, FVD]], 384)
                zc = wk.tile([TP, FVD * G], F32, tag="zc")
                zcv = _ap(zc, [[FVD, G], [1, FVD]])
                Vv.tensor_scalar(out=zcv, in0=zsl, scalar1=float(EPS),
                                 scalar2=None, op0=Alu.max)
                rz = wk.tile([TP, FVD * G], F32, tag="rz")
                rzv = _ap(rz, [[FVD, G], [1, FVD]])
                Vv.reciprocal(out=rzv, in_=zcv)

                xyt = io.tile([TP, FVD2 * G], F32, tag="xyt")
                Vv.tensor_tensor(out=_ap(xyt, [[FVD2, G], [2, FVD]]),
                                 in0=_ap(pall, [[576, G], [1, FVD]]),
                                 in1=rzv, op=Alu.mult)
                Vv.tensor_tensor(out=_ap(xyt, [[FVD2, G], [2, FVD]], 1),
                                 in0=_ap(pall, [[576, G], [1, FVD]], 192),
                                 in1=rzv, op=Alu.mult)

                mzt = wk.tile([TP, FVD * G], F32, tag="zc")
                mzv = _ap(mzt, [[FVD, G], [1, FVD]])
                Vv.tensor_scalar(out=mzv, in0=zsl, scalar1=float(EPS),
                                 scalar2=None, op0=Alu.is_gt)

                mt1 = big.tile([TP, FVD2 * G], F32, tag="bigC")
                mt2 = big.tile([TP, FVD2 * G], F32, tag="bigD")
                Vv.tensor_scalar(out=mt1, in0=xyt, scalar1=0.0, scalar2=None,
                                 op0=Alu.is_ge)
                wh_b = _ap(cn, [[0, G], [0, FVD], [1, 2]], _CWH)
                Vv.tensor_tensor(out=_ap(mt2, [[FVD2, G], [2, FVD], [1, 2]]),
                                 in0=_ap(xyt, [[FVD2, G], [2, FVD], [1, 2]]),
                                 in1=wh_b, op=Alu.is_lt)
                Vv.tensor_tensor(out=mt1, in0=mt1, in1=mt2, op=Alu.mult)
                maskf = wk.tile([TP, FVD * G], F32, tag="rz")
                mfv = _ap(maskf, [[FVD, G], [1, FVD]])
                Vv.tensor_tensor(out=mfv, in0=_ap(mt1, [[FVD2, G], [2, FVD]]),
                                 in1=_ap(mt1, [[FVD2, G], [2, FVD]], 1),
                                 op=Alu.mult)
                Vv.tensor_tensor(out=mfv, in0=mfv, in1=mzv, op=Alu.mult)

                mco = io.tile([TP, MCW * G], U8, tag="mco")
                Vv.tensor_copy(out=_ap(mco, [[MCW, G], [1, FVD]]), in_=mfv)

                # ---- LM / HM ----
                lmt = wk.tile([TP, FVD * G], F32, tag="lmt")
                Vv.tensor_scalar(out=lmt, in0=maskf, scalar1=-33.0,
                                 scalar2=None, op0=Alu.mult)
                d33_b = _ap(cn, [[0, G], [0, 6], [1, D]], _CD33)
                lmtv = _ap(lmt, [[FVD, G], [D, 6], [1, D]])
                Vv.tensor_tensor(out=lmtv, in0=lmtv, in1=d33_b, op=Alu.add)
                lm = wk.tile([TP, 6 * G], F32, tag="lm")
                Vv.tensor_reduce(out=lm, in_=lmtv, axis=mybir.AxisListType.X,
                                 op=Alu.min)
                hmt = wk.tile([TP, FVD * G], F32, tag="lmt")
                d1_b = _ap(cn, [[0, G], [0, 6], [1, D]], _CD1)
                hmtv = _ap(hmt, [[FVD, G], [D, 6], [1, D]])
                Vv.tensor_tensor(out=hmtv, in0=_ap(
                    maskf, [[FVD, G], [D, 6], [1, D]]), in1=d1_b, op=Alu.mult)
                hm = wk.tile([TP, 6 * G], F32, tag="hm")
                Vv.tensor_reduce(out=hm, in_=hmtv, axis=mybir.AxisListType.X,
                                 op=Alu.max)

                # ---- A, B line coefficients on DVE ----
                # acf block (pt, bnd, {u, w2, ub, w2b}, v):
                #   A = u + c*w2,  B = ub + c*w2b   (c = ctab boundary value)
                ctab_b = _ap(ctab, [[0, G], [1, FB]])

                def acf_b(c4):
                    return _ap(inr, [[24, 4 * G], [1, 6], [0, P]],
                               36 * G + 6 * c4)

                t1 = big.tile([*BX], F32, tag="bigA")
                Vv.tensor_tensor(out=t1, in0=ctab_b, in1=acf_b(1),
                                 op=Alu.mult)
                As = big.tile([*BX], F32, tag="bigB")
                Vv.tensor_tensor(out=As, in0=t1, in1=acf_b(0), op=Alu.add)
                t2 = big.tile([*BX], F32, tag="bigA")
                Vv.tensor_tensor(out=t2, in0=ctab_b, in1=acf_b(3),
                                 op=Alu.mult)
                Bs = big.tile([*BX], F32, tag="bigC")
                Vv.tensor_tensor(out=Bs, in0=t2, in1=acf_b(2), op=Alu.add)

                # ---- exact boundary counts ----
                s8 = big.tile([*BX], U8, tag="s8")
                Vv.tensor_scalar(out=s8, in0=As, scalar1=0.0, scalar2=None,
                                 op0=Alu.is_ge)
                rA = big.tile([*BX], F32, tag="bigD")
                Vv.reciprocal(out=rA, in_=As)
                tb = big.tile([*BX], F32, tag="bigA")
                Vv.tensor_tensor(out=tb, in0=Bs, in1=rA, op=Alu.mult)
                wv = big.tile([*BX], F32, tag="bigC")
                Vv.tensor_scalar(out=wv, in0=tb, scalar1=float(C1),
                                 scalar2=float(C2), op0=Alu.mult, op1=Alu.add)
                wc = big.tile([*BX], F32, tag="bigD")
                Vv.tensor_scalar(out=wc, in0=wv, scalar1=0.0, scalar2=1e6,
                                 op0=Alu.max, op1=Alu.min)
                q = big.tile([*BX], F32, tag="bigB")
                Sc.activation(out=q, in_=wc,
                              func=mybir.ActivationFunctionType.Sqrt)
                hp = big.tile([*BX], F32, tag="bigD")
                Vv.tensor_scalar(out=hp, in0=q, scalar1=1.0, scalar2=0.5,
                                 op0=Alu.subtract, op1=Alu.mult)
                h32 = big.tile([*BX], I32, tag="bigA")
                Vv.tensor_copy(out=h32, in_=hp)
                hf = big.tile([*BX], F32, tag="bigB")
                Vv.tensor_copy(out=hf, in_=h32)
                oh2 = big.tile([*BX], F32, tag="bigD")
                Vv.tensor_scalar(out=oh2, in0=hf, scalar1=2.0, scalar2=1.0,
                                 op0=Alu.mult, op1=Alu.add)
                bks = big.tile([*BX], F32, tag="bigA")
                Vv.tensor_tensor(out=bks, in0=oh2, in1=oh2, op=Alu.mult)
                cge = big.tile([*BX], F32, tag="bigD")
                Vv.tensor_tensor(out=cge, in0=wv, in1=bks, op=Alu.is_ge)
                cnt = big.tile([*BX], F32, tag="bigC")
                Vv.tensor_tensor(out=cnt, in0=hf, in1=cge, op=Alu.add)
                lo = big.tile([*BX], F32, tag="bigA")
                Vv.memset(lo[:, :], 0.0)
                Vv.copy_predicated(out=lo, mask=s8, data=cnt)
                c32_b = _ap(cn, [[0, FB * G]], _C32)
                Vv.copy_predicated(out=cnt, mask=s8, data=c32_b)

                # ---- combine ----
                lof = wk.tile([TP, FP * G], F32, tag="lof")
                Vv.tensor_reduce(
                    out=lof, in_=_ap(lo, [[FB, G], [1, FP], [FP, 4]]),
                    axis=mybir.AxisListType.X, op=Alu.max)
                lm_b = _ap(lm, [[6, G], [1, 6], [0, P]])
                lofv = _ap(lof, [[FP, G], [P, 6], [1, P]])
                Vv.tensor_tensor(out=lofv, in0=lofv, in1=lm_b, op=Alu.max)
                hif = wk.tile([TP, FP * G], F32, tag="hif")
                Vv.tensor_reduce(
                    out=hif, in_=_ap(cnt, [[FB, G], [1, FP], [FP, 4]]),
                    axis=mybir.AxisListType.X, op=Alu.min)
                hm_b = _ap(hm, [[6, G], [1, 6], [0, P]])
                hifv = _ap(hif, [[FP, G], [P, 6], [1, P]])
                Vv.tensor_tensor(out=hifv, in0=hifv, in1=hm_b, op=Alu.min)

                Vv.tensor_tensor(out=_ap(mco, [[MCW, G], [1, FP]], FVD),
                                 in0=lof, in1=hif, op=Alu.is_lt)
                nc.sync.dma_start(
                    out=_dap(mco_d, r0 * MCW, MCW * G, [[1, MCW * G]]),
                    in_=mco)
                nc.sync.dma_start(
                    out=_dap(xy_d, r0 * FVD2, FVD2 * G, [[1, FVD2 * G]]),
                    in_=xyt)

    nc.finalize()
    return nc


def _host_tables(trans_mats, rois_pad):
    tm = np.asarray(trans_mats, dtype=np.float32)
    rois = np.asarray(rois_pad, dtype=np.float32)

    # ctab[(bnd, v, p)] = boundary value
    specs = [(0, True, 0), (2, False, 0), (1, True, 1), (3, False, 1)]
    crow = np.zeros((4, V, P), np.float32)
    for bnd, (ci, _, _) in enumerate(specs):
        crow[bnd] = rois[:, :, ci]
    ctab = np.tile(crow.reshape(1, FB), (128, 1)).astype(np.float32)

    idxf = np.arange(D, dtype=np.float32)
    dep = (S32 + BIN * idxf * (idxf + 1)).astype(np.float32)

    row = np.zeros(CW, np.float32)
    row[_CD:_CD + D] = dep
    row[_CD33:_CD33 + D] = idxf + 33.0
    row[_CD1:_CD1 + D] = idxf + 1.0
    row[_CWH] = W2D
    row[_CWH + 1] = H2D
    row[_C32] = 32.0
    consts = np.tile(row[None, :], (128, 1)).astype(np.float32)
    return ctab, consts


def _host_points(points, trans_mats):
    """Per-point gathered transforms + line-coef features, padded/tiled."""
    tm = np.asarray(trans_mats, dtype=np.float32)
    pts = np.asarray(points, dtype=np.float32)
    n = pts.shape[0]
    g = pts[:, 0].astype(np.int32)
    x = pts[:, 1]
    y = pts[:, 2]

    M = tm[:, g]                               # [V, n, 4, 4]

    # a_i = M[i,0]*x + M[i,1]*y + M[i,2],  b_i = M[i,3]   [n, V, 3]
    a = (M[:, :, 0:3, 0] * x[None, :, None]
         + M[:, :, 0:3, 1] * y[None, :, None]
         + M[:, :, 0:3, 2]).astype(np.float32).transpose(1, 0, 2)
    b = M[:, :, 0:3, 3].transpose(1, 0, 2)     # [n, V, 3]

    # acf[(bnd, {u, w2, ub, w2b}, v)]:
    #   gt (sgn=+1): A = a_i - c*a2, B = c*b2 - b_i
    #   lt (sgn=-1): A = c*a2 - a_i, B = b_i - c*b2
    # A = u + c*w2 with u = sgn*a_i, w2 = -sgn*a2
    # B = ub + c*w2b with ub = -sgn*b_i, w2b = sgn*b2
    acf = np.zeros((n, 4, 4, V), np.float32)
    specs = [(0, True, 0), (2, False, 0), (1, True, 1), (3, False, 1)]
    for bnd, (_, gt, ai) in enumerate(specs):
        sgn = np.float32(1.0) if gt else np.float32(-1.0)
        acf[:, bnd, 0] = sgn * a[:, :, ai]
        acf[:, bnd, 1] = -sgn * a[:, :, 2]
        acf[:, bnd, 2] = -sgn * b[:, :, ai]
        acf[:, bnd, 3] = sgn * b[:, :, 2]
    acf = acf.reshape(n, ACW)
    a18 = np.ascontiguousarray(a.transpose(0, 2, 1)).reshape(n, 18)
    b18 = np.ascontiguousarray(b.transpose(0, 2, 1)).reshape(n, 18)
    return a18, b18, acf


def kernel(points, trans_mats, rois_pad):
    points = np.asarray(points, dtype=np.float32)
    reps = int(_CACHE.get("reps", 1))
    key = ("nc", reps)
    if key not in _CACHE:
        _CACHE[key] = _build_nc(reps)
    nc = _CACHE[key]

    ctab, consts = _host_tables(trans_mats, rois_pad)
    a18, b18, acf = _host_points(points, trans_mats)

    in_maps = []
    for c in range(NCORES):
        sl = slice(c * NL, (c + 1) * NL)
        ac = np.zeros((NLP, 18), np.float32)
        bc = np.zeros((NLP, 18), np.float32)
        acfp = np.zeros((NLP, ACW), np.float32)
        ac[:NL] = a18[sl]
        bc[:NL] = b18[sl]
        acfp[:NL] = acf[sl]
        acfp[NL:] = PADROW
        inr = np.concatenate([ac.reshape(NLP // G, 18 * G),
                              bc.reshape(NLP // G, 18 * G),
                              acfp.reshape(NLP // G, ACW * G)], axis=1)
        in_maps.append({"inr": np.ascontiguousarray(inr), "ctab": ctab,
                        "consts": consts})

    res = run_bass_kernel_spmd(nc, in_maps, core_ids=list(range(NCORES)))
    _CACHE["last"] = res
    outs = res.results

    xy = np.concatenate([r["xy"][:NL] for r in outs], axis=0)
    mco = np.concatenate([r["mco"][:NL] for r in outs], axis=0)
    xy = xy.reshape(N, V, D, 2)
    mask = mco[:, :FVD].reshape(N, V, D).astype(np.bool_)
    corr = mco[:, FVD:].reshape(N, V, P).astype(np.bool_)
    return xy, mask, corr
